# revision 56
# baseline (speedup 1.0000x reference)
"""Trainium2 Bass kernel for nn_MihGNNEmbeddingTest3 (gnn_message_passing).

Reference math:
    H = mlp(A_s @ emb)          (mlp = 3 linear layers, no activations)
    out[e] = relu(|<H[src_e], H[dst_e]>| / (||H[src_e]|| ||H[dst_e]||))

The mlp is affine, so fold it: H = A_s @ E2 + b_eff with E2 = emb @ W_eff^T
precomputed on host.  Default path (ROWGATHER + RG_PRE, ~88-94us, no
collectives, no device gathers):

- Data-parallel over edges per the sharding hint: each core receives the
  host-pre-gathered [2*EPC, N] A-row blocks for its own 2048 edge
  endpoints ("the gathered [B/M, N] blocks"), fp8e4(A - 0.5) packed as
  int16 k-pairs, chunk-major [128, 16, 32, 128] so every 128-endpoint
  chunk is one contiguous 1MB sequential load on the sync HWDGE ring.
  No h AllGather and no edge gather exist at all - decisive because the
  CC mesh has a fixed ~77us post-launch init here, and device-side
  dma_gather runs at only ~200GB/s/core with a ~19us Q7 init.
- Matmuls run in plain DoubleRow fp8 perf mode (RG_DR=1 default): host
  ships plane-separated pairs [128, ch, t, 2, 128] so lhsT is a natural
  [128, 2, 128] slice; rhs = E2 fp8 pre-swizzled [128, 32, 2, D] to the
  k map k = 2*(t*128+p)+pair, loaded as 3 separate tiles on the scalar
  ring (plane 0 first; the big pieces issue after chunks 0/1 so they
  don't race the gating loads for the shared DMA engines - separate
  tiles, because multi-writer sub-ranges of one tile get miswired deps
  -> NaN).  512 matmuls, ~125ns steady (LDW-bound), ~1 gap.
  RG_DR=0 keeps the SwInterleave variant (pair-interleaved rows, manual
  lhsT AP [[1,2],[2,128]], host-reversed columns) - same speed.
- Chunks interleave [src block j, dst block j] so each block's dot/norm
  DVE+ACT math runs as soon as its pair of h tiles lands; the final
  sqrt/recip/|dot| chain + out store trail the last matmul by ~3us.
- Host sorts each core's edges by src and unpermutes the output (free).
  Accuracy ~1.5e-3 median / ~1e-2 max rel err vs the 2e-2 gate.

Fallbacks: RG_PRE=0 gathers A-rows on device via dma_gather(transpose=
True) (~96-110us; per-call limit 2MB OK / 4MB HANGS the device).
ROWGATHER=0 is the AllGather design (~148-170us): fp8 DoubleRow shard
matmuls, ONE AllGather (AG_CHUNKS=8), per-side whole dma_gather,
interleaved bf16 edge math.  MM_FP8=0 falls back to bf16 matmuls.

Known-broken in this axon environment (measured, do not enable): NORM=1
and EDGE_TTR=1 (any accum_out fused reduction) hang; PREP_GATHER (SWDGE
prepare_only+trigger_dma, any queue) never fires (also sinks remote_dma);
AG_MODE=scatter and H_SHARED=1 hang or run 2x slower; the first SWDGE op
on the Q7 stalls ~14us after engine preamble (fixed init; a warm-up
SWDGE op only ADDS latency).
"""

import os
import sys

import numpy as np

try:
    import concourse.bass  # noqa: F401
except ImportError:  # pragma: no cover - grading env should have PYTHONPATH set
    for p in ("/opt/trn_rl_repo", "/root/.axon_site/_ro/trn_rl_repo"):
        if os.path.isdir(p) and p not in sys.path:
            sys.path.insert(0, p)

import ml_dtypes

N, D, B = 8192, 256, 8192
N_CORES = 8
ROWS = N // N_CORES     # A_s rows / nodes per core
EPC = B // N_CORES      # edges per core
KT = N // 128           # contraction tiles
MT = ROWS // 128        # output row m-tiles per core (8)
JT = EPC // 128         # edge blocks per core (8)
NG = 4                  # m-groups / AllGather chunks
MPG = MT // NG          # m-tiles per group (2)
GROWS = MPG * 128       # rows per group per core (256)

_CACHE = {}
LAST_RESULTS = None  # BassKernelResults of the most recent run (for test.py)


def _ind_gathers(nc, bass, hs_all, hd_all, h_full, sidx_sb, didx_sb):
    gb = int(os.environ.get("GATHER_BATCH", "1"))
    for j0 in range(0, JT, gb):
        j1 = min(j0 + gb, JT)
        nc.gpsimd.indirect_dma_start(
            out=hs_all[:, j0:j1, :],
            out_offset=None,
            in_=h_full[:],
            in_offset=bass.IndirectOffsetOnAxis(ap=sidx_sb[:, j0:j1], axis=0),
        )
        nc.gpsimd.indirect_dma_start(
            out=hd_all[:, j0:j1, :],
            out_offset=None,
            in_=h_full[:],
            in_offset=bass.IndirectOffsetOnAxis(ap=didx_sb[:, j0:j1], axis=0),
        )


def _build():
    import concourse.bacc as bacc
    import concourse.bass as bass
    import concourse.mybir as mybir
    import concourse.tile as tile

    fp32 = mybir.dt.float32
    bf16 = mybir.dt.bfloat16

    prep_gather = os.environ.get("PREP_GATHER", "0") == "1"
    nc = bacc.Bacc(
        num_devices=N_CORES,
        num_swdge_queues=int(os.environ.get("NSWQ", "1")),
    )
    fp8 = mybir.dt.float8e4
    a_fp8 = os.environ.get("A_FP8", "0") == "1"
    mm_fp8 = os.environ.get("MM_FP8", "1") == "1"
    st_eng = nc.sync if os.environ.get("NOSCALARDMA") == "1" else nc.scalar
    use_norm = os.environ.get("NORM") == "1"
    # partition-major layouts: [p, k_tile, cols]; one DRAM param per m-group
    # so each group's data is one contiguous span.
    # mm_fp8: A and E2 both fp8e4, loaded directly (no cast-DMA) and fed to
    # the PE in DoubleRow perf mode (2 k-planes per matmul, 2x throughput).
    if mm_fp8:
        at_dt = fp8
    else:
        at_dt = mybir.dt.float8e4 if a_fp8 else bf16
    e2_dt = fp8 if mm_fp8 else bf16
    _rg = os.environ.get("ROWGATHER", "1") == "1"
    ats = [
        nc.declare_dram_parameter(f"at{g}", [128, KT, GROWS], at_dt, isOutput=False)
        for g in range(NG)
    ] if not _rg else None
    e2 = (nc.declare_dram_parameter("e2", [128, KT, D], e2_dt, isOutput=False)
          if not _rg else None)
    rowgather = _rg
    if rowgather:
        # A-rows per edge endpoint, fp8(A-0.5) viewed as int16 k-pairs;
        # either gathered on device (transpose dma_gather) or pre-gathered
        # on host (rg_pre, the sharding hint's "[B/M, N] blocks").
        rg_pre = os.environ.get("RG_PRE", "1") == "1"
        rg_dr = os.environ.get("RG_DR", "1") == "1"
        if not rg_pre:
            afull = nc.declare_dram_parameter(
                "afull", [N, N // 2], mybir.dt.int16, isOutput=False)
            ridx16 = nc.declare_dram_parameter(
                "ridx16", [128, 2 * EPC // 16], mybir.dt.int16, isOutput=False)
        e2dr = nc.declare_dram_parameter(
            "e2dr", [128, KT // 2, 2, D], fp8, isOutput=False)
        if rg_pre:
            # host pre-gathers the [2*EPC, N] A-row blocks (the sharding hint's
            # "gathered [B/M, N] blocks") chunk-major so each 1-tile chunk is a
            # contiguous 1MB sequential load on the sync HWDGE ring.
            if rg_dr:
                # plane-separated pairs: plain DoubleRow, natural AP slices
                arows = nc.declare_dram_parameter(
                    "arows", [128, 2 * EPC // 128, KT // 2, 2, 128],
                    fp8, isOutput=False)
            else:
                arows = nc.declare_dram_parameter(
                    "arows", [128, 2 * EPC // 128, KT // 2, 128],
                    mybir.dt.int16, isOutput=False)
    sidx16 = nc.declare_dram_parameter(
        "sidx16", [128, EPC // 16], mybir.dt.int16, isOutput=False)
    didx16 = nc.declare_dram_parameter(
        "didx16", [128, EPC // 16], mybir.dt.int16, isOutput=False)
    bias = nc.declare_dram_parameter("bias", [128, D], fp32, isOutput=False)
    scidx = nc.declare_dram_parameter("scidx", [128, MT], mybir.dt.int32, isOutput=False)
    hofs = nc.declare_dram_parameter("hofs", [1, 2], mybir.dt.int32, isOutput=False)
    sidx = nc.declare_dram_parameter("sidx", [128, JT], mybir.dt.int32, isOutput=False)
    didx = nc.declare_dram_parameter("didx", [128, JT], mybir.dt.int32, isOutput=False)
    out = nc.declare_dram_parameter("out", [128, JT], fp32, isOutput=True)

    with tile.TileContext(nc) as tc:
        with (
            tc.tile_pool(name="atp", bufs=1) as atp,
            tc.tile_pool(name="e2p", bufs=1) as e2p,
            tc.tile_pool(name="psum", bufs=MT, space="PSUM") as psum,
            tc.tile_pool(name="hwork", bufs=4) as hwork,
            tc.tile_pool(name="dram", bufs=1, space="DRAM") as dram,
            tc.tile_pool(name="const", bufs=1) as constp,
            tc.tile_pool(name="gat", bufs=1) as gat,
            tc.tile_pool(name="small", bufs=1) as small,
        ):
            ag_mode = os.environ.get("AG_MODE", "cc2")
            use_scatter = ag_mode == "scatter"
            h_shard = dram.tile([ROWS, D], bf16)  # legacy modes stay bf16
            # uneven chunks (m-tiles 0-1 / 2-3 / 4-7): the first small AG
            # starts right after group 0 so the CC pipeline opens ~14us
            # earlier; the last doorbell time is unchanged.
            CH_MT = [int(x) for x in os.environ.get("AG_CHUNKS", "8").split(",")]
            assert sum(CH_MT) == MT, CH_MT
            CH_OF = [sum(CH_MT[:i]) for i in range(len(CH_MT))]   # first m-tile
            CH_TRG = [o + n - 1 for o, n in zip(CH_OF, CH_MT)]    # last m-tile
            h_fp8 = os.environ.get("H_FP8", "0") == "1"
            h_dt = fp8 if h_fp8 else bf16
            h_shg = [
                dram.tile([n * 128, D], h_dt, name=f"h_shg{i}")
                for i, n in enumerate(CH_MT)
            ]
            if os.environ.get("H_SHARED") == "1":
                h_space = "Shared"
            else:
                h_space = "Local" if (ag_mode == "cc2" or os.environ.get("H_LOCAL") == "1") else "Shared"
            h_full = dram.tile([N, D], h_dt, addr_space=h_space)
            bar_in = dram.tile([8, 16], fp32)
            bar_out = dram.tile([8 * N_CORES, 16], fp32, addr_space=h_space)

            # Small latency-critical loads FIRST so they never queue behind
            # the 16MB A stream on the sync HWDGE ring.
            bias_sb = constp.tile([128, D], fp32)
            if not (rowgather and os.environ.get("RG_PRE", "1") == "1"):
                nc.sync.dma_start(out=bias_sb[:], in_=bias[:])
            if os.environ.get("SWDGE_WARM", "0") == "1":
                swarm = constp.tile([1, 16], fp32)
                nc.gpsimd.dma_start(out=swarm[:], in_=bias[0:1, 0:16])
            if use_scatter or os.environ.get("CC_WARM", "0") == "1":
                nc.sync.dma_start(out=bar_in[:], in_=bias_sb[0:8, 0:16])
            if os.environ.get("CC_WARM", "0") == "1":
                # tiny AllReduce issued before the matmuls: absorbs the first
                # collective's ~40us cold-start (uCode fetch / mesh program
                # load) so the real AllGathers run at the chained rate.
                nc.gpsimd.collective_compute(
                    "AllReduce",
                    mybir.AluOpType.add,
                    replica_groups=[list(range(N_CORES))],
                    ins=[bar_in[:].opt()],
                    outs=[bar_in[:].opt()],
                )
            use_dg = os.environ.get("GATHER_MODE", "dg") == "dg"
            sidx_sb = constp.tile([128, JT], mybir.dt.int32)
            didx_sb = constp.tile([128, JT], mybir.dt.int32)
            if not use_dg:
                nc.sync.dma_start(out=sidx_sb[:], in_=sidx[:])
                nc.sync.dma_start(out=didx_sb[:], in_=didx[:])
            scidx_sb = constp.tile([128, MT], mybir.dt.int32)
            hofs_sb = constp.tile([1, 2], mybir.dt.int32)
            sidx16_sb = constp.tile([128, EPC // 16], mybir.dt.int16)
            didx16_sb = constp.tile([128, EPC // 16], mybir.dt.int16)

            # Batched loads: few big DMAs. Small leading chunks so the first
            # matmuls start early; e2 and group-0 A interleave so k-tiles
            # arrive in lockstep. All bounds even so fp8 DoubleRow k-pairs
            # never straddle a chunk boundary.
            AT0_BOUNDS = [0, 2, 6, 14, 30, 64]
            E2_BOUNDS = [0, 2, 6, 14, 30, 64]
            ATN_BOUNDS = [0, 22, 44, 64] if mm_fp8 else [0, 22, 43, 64]
            at_t = [[None] * KT for _ in range(NG)]  # [group][k] -> AP [128, GROWS]
            e2_t = [None] * KT
            # fp8 pair APs: [group][k//2] -> AP [128, 2, GROWS] / [128, 2, D]
            at_p = [[None] * (KT // 2) for _ in range(NG)]
            e2_p = [None] * (KT // 2)

            def load_e2(lo, hi):
                if rowgather:
                    return
                ec = e2p.tile(
                    [128, hi - lo, D], e2_dt, name=f"e2c_{lo}", tag=f"e2c{lo}")
                nc.sync.dma_start(out=ec[:], in_=e2[:, lo:hi, :])
                for k in range(lo, hi):
                    e2_t[k] = ec[:, k - lo, :]
                if mm_fp8:
                    for k in range(lo, hi, 2):
                        e2_p[k // 2] = ec[:, k - lo:k - lo + 2, :]

            def load_at(g, lo, hi):
                if rowgather:
                    return
                ac = atp.tile(
                    [128, hi - lo, GROWS], at_dt if mm_fp8 else bf16,
                    name=f"atc_{g}_{lo}", tag=f"atc{g}_{lo}",
                )
                if a_fp8 and not mm_fp8:
                    # SWDGE cast-DMA: fp8 in HBM -> bf16 in SBUF (halves HBM read)
                    nc.gpsimd.dma_start(out=ac[:], in_=ats[g][:, lo:hi, :])
                else:
                    nc.sync.dma_start(out=ac[:], in_=ats[g][:, lo:hi, :])
                for k in range(lo, hi):
                    at_t[g][k] = ac[:, k - lo, :]
                if mm_fp8:
                    for k in range(lo, hi, 2):
                        at_p[g][k // 2] = ac[:, k - lo:k - lo + 2, :]

            for ci in range(len(AT0_BOUNDS) - 1):
                load_e2(E2_BOUNDS[ci], E2_BOUNDS[ci + 1])
                load_at(0, AT0_BOUNDS[ci], AT0_BOUNDS[ci + 1])
                if ci == 0:
                    if use_dg and not rowgather:
                        nc.sync.dma_start(out=sidx16_sb[:], in_=sidx16[:])
                        nc.sync.dma_start(out=didx16_sb[:], in_=didx16[:])
                    if os.environ.get("AG_MODE", "cc2") == "scatter":
                        nc.sync.dma_start(out=scidx_sb[:], in_=scidx[:])
                        nc.sync.dma_start(out=hofs_sb[:], in_=hofs[:])
            for g in range(1, NG):
                for ci in range(len(ATN_BOUNDS) - 1):
                    load_at(g, ATN_BOUNDS[ci], ATN_BOUNDS[ci + 1])

            hs_all = gat.tile([128, JT, D], h_dt, name="hs_all", tag="hs_all")
            hd_all = gat.tile([128, JT, D], h_dt, name="hd_all", tag="hd_all")

            # tiny early sqrt: hoists the Sqrt ACT-table load out of the
            # edge-phase critical tail (Square alone would pick a table set
            # without Sqrt, forcing a 1.3us reload right before the output)
            warm = small.tile([128, 2], fp32, name="warm", tag="warm")
            nc.scalar.sqrt(warm[:, 0:1], bias_sb[:, 0:1])
            # node norms: ss/st/inv columns per m-tile
            ss = small.tile([128, MT], fp32, name="ss", tag="ss")
            st = small.tile([128, MT], fp32, name="st", tag="st")
            inv = small.tile([128, MT], fp32, name="inv", tag="inv")
            out_sb = constp.tile([128, JT], fp32)

            scatters = []
            hb_all = gat.tile([128, MT, D], h_dt, name="hb_all", tag="hb_all")
            if rowgather:
                NT = 2 * EPC // 128          # 16 node tiles (src || dst)
                RG_CHUNKS = [int(x) for x in os.environ.get(
                    "RG_CHUNKS", "1,1,1,1,1,1,1,1,1,1,1,1,1,1,1,1").split(",")]
                assert sum(RG_CHUNKS) == NT, RG_CHUNKS
                NCH = len(RG_CHUNKS)
                KT16 = KT // 2               # 32 int16 k-planes
                E2_SPLITS = ((0, 2), (2, 8), (8, KT16))
                e2dr_sbs = [
                    e2p.tile([128, hi - lo, 2, D], fp8, name=f"e2dr_sb{lo}")
                    for lo, hi in E2_SPLITS
                ]

                def e2p_slice(t):
                    for (lo, hi), tl in zip(E2_SPLITS, e2dr_sbs):
                        if lo <= t < hi:
                            return tl[:, t - lo, :, :]
                ridx_sb = constp.tile([128, 2 * EPC // 16], mybir.dt.int16)
                # ridx first (gates the first gather); bulk e2dr on the
                # scalar ring so it never queues ahead of it.
                if not rg_pre:
                    st_eng.dma_start(out=ridx_sb[:], in_=ridx16[:])
                # e2dr on the scalar ring, piece 0 first; the big pieces are
                # issued after chunks 0/1 below so they don't race the loads
                # that gate the first matmuls for the shared DMA engines.
                st_eng.dma_start(
                    out=e2dr_sbs[0][:], in_=e2dr[:, 0:2, :, :])
                if not rg_pre:
                    st_eng.dma_start(out=e2dr_sbs[1][:], in_=e2dr[:, 2:8, :, :])
                    st_eng.dma_start(out=e2dr_sbs[2][:], in_=e2dr[:, 8:KT16, :, :])
                rg_dot = small.tile([128, JT], fp32, name="rgdot", tag="rgdot")
                rg_ns = small.tile([128, JT], fp32, name="rgns", tag="rgns")
                rg_nd = small.tile([128, JT], fp32, name="rgnd", tag="rgnd")
                rg_edt = bf16 if os.environ.get("EDGE_BF16", "1") == "1" else fp32
                rg_prod = gat.tile([128, JT, D], rg_edt, name="rgprod", tag="rgprod")
                rg_sqs = gat.tile([128, JT, D], rg_edt, name="rgsqs", tag="rgsqs")
                rg_sqd = gat.tile([128, JT, D], rg_edt, name="rgsqd", tag="rgsqd")
                with nc.named_scope("matmul"):
                    ps_t = [
                        psum.tile([128, D], fp32, name=f"ps_{m}", tag="ps")
                        for m in range(min(MT, NT))
                    ]
                    ch_of = [sum(RG_CHUNKS[:i]) for i in range(NCH)]
                    for ch in range(NCH):
                        CT = RG_CHUNKS[ch]
                        npc = CT * 128       # idxs per chunk
                        if rg_pre and rg_dr:
                            ag_sb = hwork.tile(
                                [128, KT16, 2, npc], fp8,
                                name=f"ag_{ch}", tag=f"ag{CT}")
                        else:
                            ag_sb = hwork.tile(
                                [128, KT16, npc], mybir.dt.int16,
                                name=f"ag_{ch}", tag=f"ag{CT}")
                        if rg_pre:
                            assert CT == 1, "rg_pre ships 1-tile chunks"
                            nc.sync.dma_start(
                                out=ag_sb[:],
                                in_=(arows[:, ch_of[ch], :, :, :] if rg_dr
                                     else arows[:, ch_of[ch], :, :]))
                            if ch == 0:
                                st_eng.dma_start(
                                    out=e2dr_sbs[1][:],
                                    in_=e2dr[:, 2:8, :, :])
                                # bias on the scalar ring, emitted before
                                # tile 0's bias-add; keeps chunk 0 as the
                                # sync ring's very first transfer
                                st_eng.dma_start(
                                    out=bias_sb[:], in_=bias[:])
                            elif ch == 1:
                                st_eng.dma_start(
                                    out=e2dr_sbs[2][:],
                                    in_=e2dr[:, 8:KT16, :, :])
                        else:
                            cs = slice(ch_of[ch] * 8, (ch_of[ch] + CT) * 8)
                            nc.gpsimd.dma_gather(
                                ag_sb[:, :, :], afull[:], ridx_sb[:, cs],
                                npc, npc, N // 2, transpose=True)
                        agf = (ag_sb[:] if (rg_pre and rg_dr)
                               else ag_sb[:].bitcast(fp8))
                        if os.environ.get("RG_NOMM") == "1":
                            for lt in range(CT):
                                nt = ch_of[ch] + lt
                                dst = (hs_all[:, nt, :] if nt < JT
                                       else hd_all[:, nt - JT, :])
                                nc.vector.tensor_scalar(
                                    out=dst,
                                    in0=agf[:, 0, lt * 256:lt * 256 + 256],
                                    scalar1=1.0, scalar2=None,
                                    op0=mybir.AluOpType.mult)
                            continue
                        for lt in range(CT):  # matmul chains
                            nt = ch_of[ch] + lt
                            ps = ps_t[nt % len(ps_t)]
                            for t in range(KT16):
                                if rg_pre and rg_dr:
                                    # plane-separated pairs: natural slice,
                                    # plain DoubleRow
                                    lhs_ap = agf[:, t, :, lt * 128:(lt + 1) * 128]
                                    pm = mybir.MatmulPerfMode.DoubleRow
                                else:
                                    # interleaved stationary: plane stride 1
                                    # (the A/B byte pair), column stride 2,
                                    # columns shipped host-reversed
                                    # (SwInterleave contract)
                                    lhs_ap = bass.AP(
                                        agf.tensor,
                                        agf.offset + t * (2 * npc) + lt * 256,
                                        [agf.ap[0], [1, 2], [2, 128]],
                                    )
                                    pm = mybir.MatmulPerfMode.DoubleRowSwInterleave
                                nc.tensor.matmul(
                                    out=ps[:],
                                    lhsT=lhs_ap,
                                    rhs=e2p_slice(t),
                                    start=(t == 0),
                                    stop=(t == KT16 - 1),
                                    perf_mode=pm,
                                )
                            dst = (hs_all[:, nt // 2, :] if nt % 2 == 0
                                   else hd_all[:, nt // 2, :])
                            nc.vector.tensor_tensor(
                                out=dst, in0=ps[:], in1=bias_sb[:],
                                op=mybir.AluOpType.add,
                            )
                            if nt % 2 == 1:
                                jb = nt // 2
                                hs_b = hs_all[:, jb, :]
                                hd_b = hd_all[:, jb, :]
                                nc.vector.tensor_tensor(
                                    out=rg_prod[:, jb, :], in0=hs_b, in1=hd_b,
                                    op=mybir.AluOpType.mult)
                                nc.scalar.square(rg_sqs[:, jb, :], hs_b)
                                nc.scalar.square(rg_sqd[:, jb, :], hd_b)
                                nc.vector.tensor_reduce(
                                    out=rg_dot[:, jb:jb + 1],
                                    in_=rg_prod[:, jb, :],
                                    axis=mybir.AxisListType.X,
                                    op=mybir.AluOpType.add)
                                nc.vector.tensor_reduce(
                                    out=rg_ns[:, jb:jb + 1],
                                    in_=rg_sqs[:, jb, :],
                                    axis=mybir.AxisListType.X,
                                    op=mybir.AluOpType.add)
                                nc.vector.tensor_reduce(
                                    out=rg_nd[:, jb:jb + 1],
                                    in_=rg_sqd[:, jb, :],
                                    axis=mybir.AxisListType.X,
                                    op=mybir.AluOpType.add)
                with nc.named_scope("rgchain"):
                    nsnd = small.tile([128, JT], fp32, name="nsnd", tag="nsnd")
                    nc.vector.tensor_tensor(
                        out=nsnd[:], in0=rg_ns[:], in1=rg_nd[:],
                        op=mybir.AluOpType.mult)
                    stq = small.tile([128, JT], fp32, name="stq", tag="stq")
                    nc.scalar.sqrt(stq[:], nsnd[:])
                    invq = small.tile([128, JT], fp32, name="invq", tag="invq")
                    nc.vector.reciprocal(invq[:], stq[:])
                    ad = small.tile([128, JT], fp32, name="ad", tag="ad")
                    nc.vector.tensor_scalar(
                        out=ad[:].bitcast(mybir.dt.uint32),
                        in0=rg_dot[:].bitcast(mybir.dt.uint32),
                        scalar1=0x7FFFFFFF, scalar2=None,
                        op0=mybir.AluOpType.bitwise_and,
                    )
                    nc.vector.tensor_tensor(
                        out=out_sb[:], in0=ad[:], in1=invq[:],
                        op=mybir.AluOpType.mult)
            else:
              with nc.named_scope("matmul"):
                ps_t = [
                    psum.tile([128, D], fp32, name=f"ps_{m}", tag="ps")
                    for m in range(MT)
                ]
                ag_chunks = []
                for g in range(NG):
                    ms = range(g * MPG, (g + 1) * MPG)
                    # last group runs m-outer so m6's psum (and its store)
                    # completes ~11us before the final matmul, shortening the
                    # last AllGather's doorbell chain to m7's store alone
                    if mm_fp8:
                        KP = KT // 2
                        order = (
                            [(kp, m) for m in ms for kp in range(KP)]
                            if g == NG - 1 else
                            [(kp, m) for kp in range(KP) for m in ms]
                        )
                        for kp, m in order:
                            lm = m - g * MPG
                            nc.tensor.matmul(
                                out=ps_t[m][:],
                                lhsT=at_p[g][kp][:, :, lm * 128:(lm + 1) * 128],
                                rhs=e2_p[kp],
                                start=(kp == 0),
                                stop=(kp == KP - 1),
                                perf_mode=mybir.MatmulPerfMode.DoubleRow,
                            )
                    else:
                        order = (
                            [(k, m) for m in ms for k in range(KT)]
                            if g == NG - 1 else
                            [(k, m) for k in range(KT) for m in ms]
                        )
                        for k, m in order:
                            lm = m - g * MPG
                            nc.tensor.matmul(
                                out=ps_t[m][:],
                                lhsT=at_t[g][k][:, lm * 128:(lm + 1) * 128],
                                rhs=e2_t[k],
                                start=(k == 0),
                                stop=(k == KT - 1),
                            )
                    with nc.named_scope(f"norm{g}"):
                        for m in ms:
                            if use_norm:
                                t = hwork.tile([128, D], fp32, name=f"t_{m}", tag="t")
                                nc.vector.tensor_tensor(
                                    out=t[:], in0=ps_t[m][:], in1=bias_sb[:],
                                    op=mybir.AluOpType.add,
                                )
                                sq = hwork.tile([128, D], fp32, name=f"sq_{m}", tag="sq")
                                if os.environ.get("NORM_IMPL", "dve") == "act":
                                    nc.scalar.activation(
                                        out=sq[:], in_=t[:],
                                        func=mybir.ActivationFunctionType.Square,
                                        accum_out=ss[:, m:m + 1],
                                    )
                                else:
                                    nc.vector.tensor_tensor_reduce(
                                        out=sq[:], in0=t[:], in1=t[:],
                                        scale=1.0, scalar=0.0,
                                        op0=mybir.AluOpType.mult,
                                        op1=mybir.AluOpType.add,
                                        accum_out=ss[:, m:m + 1],
                                    )
                                nc.scalar.sqrt(st[:, m:m + 1], ss[:, m:m + 1])
                                nc.vector.reciprocal(inv[:, m:m + 1], st[:, m:m + 1])
                                hb = hb_all[:, m, :]
                                if os.environ.get("NORM_IMPL", "dve") == "act":
                                    nc.scalar.activation(
                                        out=hb, in_=t[:],
                                        func=mybir.ActivationFunctionType.Copy,
                                        bias=0.0, scale=inv[:, m:m + 1],
                                    )
                                else:
                                    nc.vector.tensor_scalar(
                                        out=hb, in0=t[:],
                                        scalar1=inv[:, m:m + 1], scalar2=None,
                                        op0=mybir.AluOpType.mult,
                                    )
                            else:
                                hb = hb_all[:, m, :]
                                if h_fp8:
                                    # h/4 in fp8e4; the global scale cancels
                                    # in cos = |dot|/(|hs||hd|).  bias_sb is
                                    # shipped pre-scaled by 1/4.
                                    nc.vector.scalar_tensor_tensor(
                                        out=hb, in0=ps_t[m][:], scalar=0.25,
                                        in1=bias_sb[:],
                                        op0=mybir.AluOpType.mult,
                                        op1=mybir.AluOpType.add,
                                    )
                                else:
                                    nc.vector.tensor_tensor(
                                        out=hb, in0=ps_t[m][:], in1=bias_sb[:],
                                        op=mybir.AluOpType.add,
                                    )
                            if ag_mode == "cc2":
                                ch = max(
                                    i for i, o in enumerate(CH_OF) if m >= o
                                )
                                lm2 = m - CH_OF[ch]
                                # SWDGE store: its completion sem does not
                                # alias the sync-ring A-load lanes, so the
                                # AllGather doorbell fires as soon as the
                                # chunk is really ready (was +18us late)
                                h_st = (nc.gpsimd if os.environ.get(
                                    "H_ENG", "gpsimd") == "gpsimd" else st_eng)
                                h_st.dma_start(
                                    out=h_shg[ch][lm2 * 128:(lm2 + 1) * 128, :],
                                    in_=hb,
                                )

                    last_m = g * MPG + MPG - 1
                    if ag_mode == "cc2" and last_m in CH_TRG:
                        ch = CH_TRG.index(last_m)
                        base = CH_OF[ch] * 128 * N_CORES
                        size = CH_MT[ch] * 128 * N_CORES
                        with nc.named_scope(f"ag{ch}"):
                            ag_chunks.append(nc.gpsimd.collective_compute(
                                "AllGather",
                                mybir.AluOpType.bypass,
                                replica_groups=[list(range(N_CORES))],
                                ins=[h_shg[ch][:]],
                                outs=[h_full[base:base + size, :]],
                            ))
                with nc.named_scope("allgather"):
                    if ag_mode == "cc2":
                        pass
                    elif use_scatter:
                        ofs = nc.sync.value_load(
                            hofs_sb[0:1, 0:1], min_val=0, max_val=(N_CORES - 1) * ROWS
                        )
                        hf = h_full[:]
                        dyn_out = bass.AP(
                            hf.tensor,
                            ofs * D,
                            [[D, 128], [128 * D, MT], [1, D]],
                        )
                        hw = nc.sync.dma_start(out=dyn_out, in_=hb_all[:])
                        if os.environ.get("NOBAR") == "1":
                            bar_cc = hw
                        else:
                            bar_cc = nc.gpsimd.collective_compute(
                                "AllReduce",
                                mybir.AluOpType.add,
                                replica_groups=[list(range(N_CORES))],
                                ins=[bar_in[:].opt()],
                                outs=[bar_in[:].opt()],
                            )
                            bass._add_dep_helper(
                                bar_cc.ins, hw.ins, sync=True,
                                reason="barrier waits for h write",
                            )
                    else:
                        for m in range(MT):
                            st_eng.dma_start(
                                out=h_shard[m * 128:(m + 1) * 128, :],
                                in_=hb_all[:, m, :],
                            )
                        bar_cc = nc.gpsimd.collective_compute(
                            "AllGather",
                            mybir.AluOpType.bypass,
                            replica_groups=[list(range(N_CORES))],
                            ins=[h_shard[:]],
                            outs=[h_full[:]],
                        )

            # SWDGE descriptor generation for the edge-row gathers.  Issued
            # after the collective doorbells (so the h_full RAW edge keeps
            # its normal writer->reader direction) but the desc-gen itself
            # (~1.3us/call on the Q7) runs under the AllGather's mesh
            # rendezvous; trigger_dma in the edges scope then only pays the
            # DMA transfer.
            QN = int(os.environ.get("GATHER_SPLIT", "1"))
            use_dg = os.environ.get("GATHER_MODE", "dg") == "dg"
            if prep_gather and use_dg:
                assert QN == 1, "prep_gather path gathers each side whole"
                gprep = nc.alloc_semaphore("gprep")
                gsem_s = nc.alloc_semaphore("gath_dma_s")
                gsem_d = nc.alloc_semaphore("gath_dma_d")
                nc.gpsimd.dma_gather(
                    hs_all[:, :, :], h_full[:], sidx16_sb[:, :], EPC, EPC, D,
                    prepare_only=True, sem=gsem_s,
                    queue_num=int(os.environ.get("PREP_Q", "0")),
                ).then_inc(gprep, 1)
                nc.gpsimd.dma_gather(
                    hd_all[:, :, :], h_full[:], didx16_sb[:, :], EPC, EPC, D,
                    prepare_only=True, sem=gsem_d,
                    queue_num=int(os.environ.get("PREP_Q", "0")),
                ).then_inc(gprep, 1)

            with nc.named_scope("edges"):
                HEPC = EPC // QN
                HJT = JT // QN
                gs = []
                interleave = (
                    use_dg and not prep_gather and QN == 1 and not use_norm
                    and os.environ.get("EDGE_TTR", "0") != "1"
                    and os.environ.get("EDGE_IL", "1") == "1"
                ) or rowgather
                if interleave:
                    pass  # gathers emitted inside the math below
                elif use_dg:
                    if prep_gather:
                        # Manual protocol (docstring pattern): wait for the
                        # descriptor writes, fire both queues' entries once
                        # the AllGather has fully written h_full, then gate
                        # the consumer engines on the DMA-completion sems.
                        nc.gpsimd.wait_ge(gprep, 2)
                        trig = nc.gpsimd.trigger_dma(
                            count=2, queue_num=int(os.environ.get("PREP_Q", "0")))
                        if ag_chunks:
                            bass._add_dep_helper(
                                trig.ins, ag_chunks[-1].ins, sync=True,
                                reason="gather transfers wait for h_full")
                        nc.vector.wait_ge(gsem_s, 16)
                        nc.vector.wait_ge(gsem_d, 16)
                        nc.scalar.wait_ge(gsem_s, 16)
                        nc.scalar.wait_ge(gsem_d, 16)
                    else:
                        for h in range(QN):
                            js = slice(h * HJT, (h + 1) * HJT)
                            cs = slice(h * (HEPC // 16), (h + 1) * (HEPC // 16))
                            gs.append(nc.gpsimd.dma_gather(
                                hs_all[:, js, :], h_full[:], sidx16_sb[:, cs],
                                HEPC, HEPC, D))
                            gs.append(nc.gpsimd.dma_gather(
                                hd_all[:, js, :], h_full[:], didx16_sb[:, cs],
                                HEPC, HEPC, D))
                        if use_scatter:
                            for gg in gs:
                                bass._add_dep_helper(
                                    gg.ins, bar_cc.ins, sync=True,
                                    reason="gathers wait for cross-core barrier")
                else:
                    _ind_gathers(nc, bass, hs_all, hd_all, h_full, sidx_sb, didx_sb)
                dot = small.tile([128, JT], fp32, name="dot", tag="dot")

                if rowgather:
                    pass
                elif interleave:
                    # hs-side square/reduce runs under the hd gather's Q7
                    # descriptor generation (~8.5us), so only the dot/nd
                    # chain remains after the second gather lands.
                    e_dt = bf16 if os.environ.get("EDGE_BF16", "1") == "1" else fp32
                    ns = small.tile([128, JT], fp32, name="ns", tag="ns")
                    nd = small.tile([128, JT], fp32, name="nd", tag="nd")
                    prod = gat.tile([128, JT, D], e_dt, name="prod", tag="prod")
                    sq_s = gat.tile([128, JT, D], e_dt, name="sq_s", tag="sq_s")
                    sq_d = gat.tile([128, JT, D], e_dt, name="sq_d", tag="sq_d")
                    if not rowgather:
                        nc.gpsimd.dma_gather(
                            hs_all[:, :, :], h_full[:], sidx16_sb[:, :],
                            EPC, EPC, D)
                    nc.scalar.square(sq_s[:], hs_all[:])
                    nc.vector.tensor_reduce(
                        out=ns[:], in_=sq_s[:],
                        axis=mybir.AxisListType.X, op=mybir.AluOpType.add)
                    if not rowgather:
                        nc.gpsimd.dma_gather(
                            hd_all[:, :, :], h_full[:], didx16_sb[:, :],
                            EPC, EPC, D)
                    nc.vector.tensor_tensor(
                        out=prod[:], in0=hs_all[:], in1=hd_all[:],
                        op=mybir.AluOpType.mult)
                    nc.scalar.square(sq_d[:], hd_all[:])
                    nc.vector.tensor_reduce(
                        out=dot[:], in_=prod[:],
                        axis=mybir.AxisListType.X, op=mybir.AluOpType.add)
                    nc.vector.tensor_reduce(
                        out=nd[:], in_=sq_d[:],
                        axis=mybir.AxisListType.X, op=mybir.AluOpType.add)
                    nsnd = small.tile([128, JT], fp32, name="nsnd", tag="nsnd")
                    nc.vector.tensor_tensor(
                        out=nsnd[:], in0=ns[:], in1=nd[:],
                        op=mybir.AluOpType.mult)
                    stq = small.tile([128, JT], fp32, name="stq", tag="stq")
                    nc.scalar.sqrt(stq[:], nsnd[:])
                    invq = small.tile([128, JT], fp32, name="invq", tag="invq")
                    nc.vector.reciprocal(invq[:], stq[:])
                    ad = small.tile([128, JT], fp32, name="ad", tag="ad")
                    nc.vector.tensor_scalar(
                        out=ad[:].bitcast(mybir.dt.uint32),
                        in0=dot[:].bitcast(mybir.dt.uint32),
                        scalar1=0x7FFFFFFF, scalar2=None,
                        op0=mybir.AluOpType.bitwise_and,
                    )
                    nc.vector.tensor_tensor(
                        out=out_sb[:], in0=ad[:], in1=invq[:],
                        op=mybir.AluOpType.mult)
                elif os.environ.get("EDGE_TTR", "0") == "1" and not use_norm:
                    # dot/ns/nd fused multiply-accumulates, one [128, D] slab
                    # per edge block, spread across DVE (dot), ACT (ns) and
                    # GpSimd (nd) so the three reductions run in parallel.
                    ns = small.tile([128, JT], fp32, name="ns", tag="ns")
                    nd = small.tile([128, JT], fp32, name="nd", tag="nd")
                    for j in range(JT):
                        pw = hwork.tile([128, D], fp32, name=f"pw_{j}", tag="pw")
                        nc.vector.tensor_tensor_reduce(
                            out=pw[:], in0=hs_all[:, j, :], in1=hd_all[:, j, :],
                            scale=1.0, scalar=0.0,
                            op0=mybir.AluOpType.mult,
                            op1=mybir.AluOpType.add,
                            accum_out=dot[:, j:j + 1])
                        qw = hwork.tile([128, D], fp32, name=f"qw_{j}", tag="qw")
                        nc.scalar.activation(
                            out=qw[:], in_=hs_all[:, j, :],
                            func=mybir.ActivationFunctionType.Square,
                            accum_out=ns[:, j:j + 1])
                        rw = hwork.tile([128, D], fp32, name=f"rw_{j}", tag="rw")
                        nc.vector.tensor_tensor_reduce(
                            out=rw[:], in0=hd_all[:, j, :], in1=hd_all[:, j, :],
                            scale=1.0, scalar=0.0,
                            op0=mybir.AluOpType.mult,
                            op1=mybir.AluOpType.add,
                            accum_out=nd[:, j:j + 1])
                    nsnd = small.tile([128, JT], fp32, name="nsnd", tag="nsnd")
                    nc.vector.tensor_tensor(
                        out=nsnd[:], in0=ns[:], in1=nd[:],
                        op=mybir.AluOpType.mult)
                    stq = small.tile([128, JT], fp32, name="stq", tag="stq")
                    nc.scalar.sqrt(stq[:], nsnd[:])
                    invq = small.tile([128, JT], fp32, name="invq", tag="invq")
                    nc.vector.reciprocal(invq[:], stq[:])
                    ad = small.tile([128, JT], fp32, name="ad", tag="ad")
                    nc.vector.tensor_scalar(
                        out=ad[:].bitcast(mybir.dt.uint32),
                        in0=dot[:].bitcast(mybir.dt.uint32),
                        scalar1=0x7FFFFFFF, scalar2=None,
                        op0=mybir.AluOpType.bitwise_and,
                    )
                    nc.vector.tensor_tensor(
                        out=out_sb[:], in0=ad[:], in1=invq[:],
                        op=mybir.AluOpType.mult)
                elif use_norm and os.environ.get("EDGE_IMPL", "new") == "new":
                    for j in range(JT):
                        prod = hwork.tile([128, D], fp32, name=f"prod_{j}", tag="prod")
                        nc.vector.tensor_tensor_reduce(
                            out=prod[:],
                            in0=hs_all[:, j, :],
                            in1=hd_all[:, j, :],
                            scale=1.0,
                            scalar=0.0,
                            op0=mybir.AluOpType.mult,
                            op1=mybir.AluOpType.add,
                            accum_out=dot[:, j:j + 1],
                        )
                    nc.scalar.activation(
                        out=out_sb[:], in_=dot[:],
                        func=mybir.ActivationFunctionType.Abs,
                    )
                else:
                    ns = small.tile([128, JT], fp32, name="ns", tag="ns")
                    nd = small.tile([128, JT], fp32, name="nd", tag="nd")
                    # bf16 intermediates double DVE throughput; the reduces
                    # still accumulate into fp32 (dot/ns/nd), so only the
                    # per-element products are rounded (~2^-8 rel, harmless
                    # next to the fp8 input quantization).
                    e_dt = bf16 if os.environ.get("EDGE_BF16", "1") == "1" else fp32
                    prod = gat.tile([128, JT, D], e_dt, name="prod", tag="prod")
                    sq_s = gat.tile([128, JT, D], e_dt, name="sq_s", tag="sq_s")
                    sq_d = gat.tile([128, JT, D], e_dt, name="sq_d", tag="sq_d")
                    for h in range(QN):
                        js = slice(h * HJT, (h + 1) * HJT)
                        nc.vector.tensor_tensor(
                            out=prod[:, js, :], in0=hs_all[:, js, :],
                            in1=hd_all[:, js, :],
                            op=mybir.AluOpType.mult,
                        )
                        nc.vector.tensor_reduce(
                            out=dot[:, js], in_=prod[:, js, :],
                            axis=mybir.AxisListType.X,
                            op=mybir.AluOpType.add,
                        )
                        nc.scalar.square(sq_s[:, js, :], hs_all[:, js, :])
                        nc.scalar.square(sq_d[:, js, :], hd_all[:, js, :])
                        nc.vector.tensor_reduce(
                            out=ns[:, js], in_=sq_s[:, js, :],
                            axis=mybir.AxisListType.X,
                            op=mybir.AluOpType.add,
                        )
                        nc.vector.tensor_reduce(
                            out=nd[:, js], in_=sq_d[:, js, :],
                            axis=mybir.AxisListType.X,
                            op=mybir.AluOpType.add,
                        )
                        nsnd = small.tile([128, JT], fp32, name="nsnd", tag="nsnd")
                        nc.vector.tensor_tensor(
                            out=nsnd[:, js], in0=ns[:, js], in1=nd[:, js],
                            op=mybir.AluOpType.mult,
                        )
                        stq = small.tile([128, JT], fp32, name="stq", tag="stq")
                        nc.scalar.sqrt(stq[:, js], nsnd[:, js])
                        invq = small.tile([128, JT], fp32, name="invq", tag="invq")
                        nc.vector.reciprocal(invq[:, js], stq[:, js])
                        ad = small.tile([128, JT], fp32, name="ad", tag="ad")
                        nc.vector.tensor_scalar(
                            out=ad[:, js].bitcast(mybir.dt.uint32),
                            in0=dot[:, js].bitcast(mybir.dt.uint32),
                            scalar1=0x7FFFFFFF, scalar2=None,
                            op0=mybir.AluOpType.bitwise_and,
                        )
                        nc.vector.tensor_tensor(
                            out=out_sb[:, js], in0=ad[:, js], in1=invq[:, js],
                            op=mybir.AluOpType.mult,
                        )

            st_eng.dma_start(out=out[:], in_=out_sb[:])

    nc.compile()
    return nc


def _get_nc():
    if "nc" not in _CACHE:
        _CACHE["nc"] = _build()
    return _CACHE["nc"]


def _remap(n):
    # node id -> h_full row. cc2 mode: two chunked AllGathers; chunk g holds
    # rows [g*512, (g+1)*512) of every core shard, concatenated rank-major.
    if os.environ.get("AG_MODE", "cc2") != "cc2":
        return n
    ch_mt = [int(x) for x in os.environ.get("AG_CHUNKS", "8").split(",")]
    ch_of = np.array([sum(ch_mt[:i]) for i in range(len(ch_mt))]) * 128
    ch_rows = np.array(ch_mt) * 128
    o = n // ROWS
    l = n % ROWS
    g = np.searchsorted(ch_of, l, side="right") - 1
    return ch_of[g] * N_CORES + o * ch_rows[g] + (l - ch_of[g])


def kernel(edges, A_s, emb, Ws, bs):
    global LAST_RESULTS
    from concourse.bass_utils import run_bass_kernel_spmd

    bf16 = ml_dtypes.bfloat16
    A = np.asarray(A_s, dtype=np.float32)
    E = np.asarray(emb, dtype=np.float32)
    W = np.asarray(Ws, dtype=np.float32)
    b = np.asarray(bs, dtype=np.float32)
    ed = np.asarray(edges)

    a_fp8 = os.environ.get("A_FP8", "0") == "1"
    mm_fp8 = os.environ.get("MM_FP8", "1") == "1"
    shift_a = a_fp8 or mm_fp8
    f8 = ml_dtypes.float8_e4m3fn
    M = W[0].T @ W[1].T @ W[2].T                      # [D, D]
    e2_np_dt = f8 if mm_fp8 else bf16
    E2f = (E @ M).astype(e2_np_dt)                    # [N, D] as used on device
    # partition-major: [128(p), KT(t), D] with row t*128+p at [p, t, :]
    E2 = np.ascontiguousarray(E2f.reshape(KT, 128, D).transpose(1, 0, 2))
    b_eff = (b[0] @ W[1].T + b[1]) @ W[2].T + b[2]    # [D]
    if shift_a:
        # A shipped as fp8(A - 0.5); fold the +0.5 row-sum term into the bias
        b_eff = b_eff + 0.5 * E2f.astype(np.float32).sum(0)
    if os.environ.get("H_FP8", "0") == "1":
        b_eff = b_eff * 0.25
    bias_rep = np.ascontiguousarray(
        np.broadcast_to(b_eff.astype(np.float32), (128, D))
    )

    rowgather = os.environ.get("ROWGATHER", "1") == "1"
    if rowgather:
        A8 = (A - 0.5).astype(f8)                     # [N, N]
        afull16 = A8.view(np.int16)                   # [N, N//2] k-pairs
        E2q32 = E2f.astype(np.float32)
        e2dr = np.ascontiguousarray(
            E2q32.astype(f8).reshape(KT // 2, 128, 2, D)
            .transpose(1, 0, 2, 3))
    in_maps = []
    perms = []
    for c in range(N_CORES):
        m = {"bias": bias_rep}
        if not rowgather:
            m["e2"] = E2
        if not rowgather:
            for g in range(NG):
                r0 = c * ROWS + g * GROWS
                blk = A[r0:r0 + GROWS, :].T               # [N, GROWS]
                blk = (blk - 0.5).astype(f8) if shift_a else blk.astype(bf16)
                m[f"at{g}"] = np.ascontiguousarray(
                    blk.reshape(KT, 128, GROWS).transpose(1, 0, 2)
                )
        e = ed[c * EPC:(c + 1) * EPC].astype(np.int64)
        if rowgather:
            order = np.argsort(e[:, 0], kind="stable")
            perms.append(order)
            e = e[order]
            m["e2dr"] = e2dr
            # blocks interleaved [src_0, dst_0, src_1, dst_1, ...] so each
            # chunk completes whole edge blocks (per-chunk dot/norm math)
            flat = np.stack(
                [e[:, 0].reshape(JT, 128), e[:, 1].reshape(JT, 128)],
                axis=1).reshape(-1)
            # reverse within each 128-block: DoubleRowSwInterleave reads the
            # stationary columns last-first, so ship them pre-reversed.
            rev = flat.reshape(-1, 128)[:, ::-1].reshape(-1)
            if os.environ.get("RG_PRE", "1") == "1":
                # host pre-gather: the sharding hint's "[B/M, N] blocks",
                # chunk-major so each device load is contiguous
                if os.environ.get("RG_DR", "1") == "1":
                    g8 = A8[flat]                     # [2*EPC, N] fp8
                    m["arows"] = np.ascontiguousarray(
                        g8.reshape(2 * EPC // 128, 128, KT // 2, 128, 2)
                        .transpose(3, 0, 2, 4, 1))    # [p, ch, t, pair, j]
                else:
                    g = afull16[rev]                  # [2*EPC, N//2] int16
                    m["arows"] = np.ascontiguousarray(
                        g.reshape(2 * EPC // 128, 128, KT // 2, 128)
                        .transpose(3, 0, 2, 1))       # [p, ch, t, j]
            else:
                m["afull"] = afull16
                m["ridx16"] = np.ascontiguousarray(
                    np.tile(rev.astype(np.int16).reshape(-1, 16).T, (8, 1)))
        m["sidx"] = np.ascontiguousarray(
            _remap(e[:, 0]).astype(np.int32).reshape(JT, 128).T
        )
        dsrc = e[:, 0] if os.environ.get("PROBE_DD_EQ_SS") == "1" else e[:, 1]
        m["didx"] = np.ascontiguousarray(
            _remap(dsrc).astype(np.int32).reshape(JT, 128).T
        )

        QN = int(os.environ.get("GATHER_SPLIT", "1"))

        def wrap16(flat):
            # QN independent gathers: wrap each EPC/QN-index chunk separately
            def w(f):
                buf = f.astype(np.int16).reshape(-1, 16).T
                return np.tile(buf, (8, 1))
            step = EPC // QN
            return np.ascontiguousarray(
                np.hstack([w(flat[q * step:(q + 1) * step]) for q in range(QN)]))

        m["hofs"] = np.array([[c * ROWS, 0]], dtype=np.int32)
        m["scidx"] = np.ascontiguousarray(
            (c * ROWS + np.arange(MT)[None, :] * 128
             + np.arange(128)[:, None]).astype(np.int32))
        m["sidx16"] = wrap16(_remap(e[:, 0]))
        m["didx16"] = wrap16(_remap(dsrc))
        in_maps.append(m)

    nc = _get_nc()
    kw = {}
    if os.environ.get("KERNEL_TRACE_KW"):
        import json
        kw = json.loads(os.environ["KERNEL_TRACE_KW"])
    res = run_bass_kernel_spmd(nc, in_maps, list(range(N_CORES)), **kw)
    LAST_RESULTS = res

    outs = []
    for c in range(N_CORES):
        oc = np.ascontiguousarray(res.results[c]["out"].T).reshape(-1)
        if rowgather:
            inv = np.empty_like(perms[c])
            inv[perms[c]] = np.arange(EPC)
            oc = oc[inv]
        outs.append(oc)
    out = np.concatenate(outs)
    return np.maximum(out, 0.0).astype(np.float32)



# revision 57
# speedup vs baseline: 1.0880x; 1.0880x over previous
"""Trainium2 Bass kernel for nn_MihGNNEmbeddingTest3 (gnn_message_passing).

Reference math:
    H = mlp(A_s @ emb)          (mlp = 3 linear layers, no activations)
    out[e] = relu(|<H[src_e], H[dst_e]>| / (||H[src_e]|| ||H[dst_e]||))

The mlp is affine, so fold it: H = A_s @ E2 + b_eff with E2 = emb @ W_eff^T
precomputed on host.  Default path (ROWGATHER + RG_PRE, ~88-94us, no
collectives, no device gathers):

- Data-parallel over edges per the sharding hint: each core receives the
  host-pre-gathered [2*EPC, N] A-row blocks for its own 2048 edge
  endpoints ("the gathered [B/M, N] blocks"), fp8e4(A - 0.5) packed as
  int16 k-pairs, chunk-major [128, 16, 32, 128] so every 128-endpoint
  chunk is one contiguous 1MB sequential load on the sync HWDGE ring.
  No h AllGather and no edge gather exist at all - decisive because the
  CC mesh has a fixed ~77us post-launch init here, and device-side
  dma_gather runs at only ~200GB/s/core with a ~19us Q7 init.
- Matmuls run in plain DoubleRow fp8 perf mode (RG_DR=1 default): host
  ships plane-separated pairs [128, ch, t, 2, 128] so lhsT is a natural
  [128, 2, 128] slice; rhs = E2 fp8 pre-swizzled [128, 32, 2, D] to the
  k map k = 2*(t*128+p)+pair, loaded as 3 separate tiles on the scalar
  ring (plane 0 first; the big pieces issue after chunks 0/1 so they
  don't race the gating loads for the shared DMA engines - separate
  tiles, because multi-writer sub-ranges of one tile get miswired deps
  -> NaN).  512 matmuls, ~125ns steady (LDW-bound), ~1 gap.
  RG_DR=0 keeps the SwInterleave variant (pair-interleaved rows, manual
  lhsT AP [[1,2],[2,128]], host-reversed columns) - same speed.
- Chunks interleave [src block j, dst block j] so each block's dot/norm
  DVE+ACT math runs as soon as its pair of h tiles lands; the final
  sqrt/recip/|dot| chain + out store trail the last matmul by ~3us.
- Host sorts each core's edges by src and unpermutes the output (free).
  Accuracy ~1.5e-3 median / ~1e-2 max rel err vs the 2e-2 gate.

Fallbacks: RG_PRE=0 gathers A-rows on device via dma_gather(transpose=
True) (~96-110us; per-call limit 2MB OK / 4MB HANGS the device).
ROWGATHER=0 is the AllGather design (~148-170us): fp8 DoubleRow shard
matmuls, ONE AllGather (AG_CHUNKS=8), per-side whole dma_gather,
interleaved bf16 edge math.  MM_FP8=0 falls back to bf16 matmuls.

Known-broken in this axon environment (measured, do not enable): NORM=1
and EDGE_TTR=1 (any accum_out fused reduction) hang; PREP_GATHER (SWDGE
prepare_only+trigger_dma, any queue) never fires (also sinks remote_dma);
AG_MODE=scatter and H_SHARED=1 hang or run 2x slower; the first SWDGE op
on the Q7 stalls ~14us after engine preamble (fixed init; a warm-up
SWDGE op only ADDS latency).
"""

import os
import sys

import numpy as np

try:
    import concourse.bass  # noqa: F401
except ImportError:  # pragma: no cover - grading env should have PYTHONPATH set
    for p in ("/opt/trn_rl_repo", "/root/.axon_site/_ro/trn_rl_repo"):
        if os.path.isdir(p) and p not in sys.path:
            sys.path.insert(0, p)

import ml_dtypes

N, D, B = 8192, 256, 8192
N_CORES = 8
ROWS = N // N_CORES     # A_s rows / nodes per core
EPC = B // N_CORES      # edges per core
KT = N // 128           # contraction tiles
MT = ROWS // 128        # output row m-tiles per core (8)
JT = EPC // 128         # edge blocks per core (8)
NG = 4                  # m-groups / AllGather chunks
MPG = MT // NG          # m-tiles per group (2)
GROWS = MPG * 128       # rows per group per core (256)

_CACHE = {}
LAST_RESULTS = None  # BassKernelResults of the most recent run (for test.py)


def _ind_gathers(nc, bass, hs_all, hd_all, h_full, sidx_sb, didx_sb):
    gb = int(os.environ.get("GATHER_BATCH", "1"))
    for j0 in range(0, JT, gb):
        j1 = min(j0 + gb, JT)
        nc.gpsimd.indirect_dma_start(
            out=hs_all[:, j0:j1, :],
            out_offset=None,
            in_=h_full[:],
            in_offset=bass.IndirectOffsetOnAxis(ap=sidx_sb[:, j0:j1], axis=0),
        )
        nc.gpsimd.indirect_dma_start(
            out=hd_all[:, j0:j1, :],
            out_offset=None,
            in_=h_full[:],
            in_offset=bass.IndirectOffsetOnAxis(ap=didx_sb[:, j0:j1], axis=0),
        )


def _build():
    import concourse.bacc as bacc
    import concourse.bass as bass
    import concourse.mybir as mybir
    import concourse.tile as tile

    fp32 = mybir.dt.float32
    bf16 = mybir.dt.bfloat16

    prep_gather = os.environ.get("PREP_GATHER", "0") == "1"
    nc = bacc.Bacc(
        num_devices=N_CORES,
        num_swdge_queues=int(os.environ.get("NSWQ", "1")),
    )
    fp8 = mybir.dt.float8e4
    a_fp8 = os.environ.get("A_FP8", "0") == "1"
    mm_fp8 = os.environ.get("MM_FP8", "1") == "1"
    st_eng = nc.sync if os.environ.get("NOSCALARDMA") == "1" else nc.scalar
    use_norm = os.environ.get("NORM") == "1"
    # partition-major layouts: [p, k_tile, cols]; one DRAM param per m-group
    # so each group's data is one contiguous span.
    # mm_fp8: A and E2 both fp8e4, loaded directly (no cast-DMA) and fed to
    # the PE in DoubleRow perf mode (2 k-planes per matmul, 2x throughput).
    if mm_fp8:
        at_dt = fp8
    else:
        at_dt = mybir.dt.float8e4 if a_fp8 else bf16
    e2_dt = fp8 if mm_fp8 else bf16
    _rg = os.environ.get("ROWGATHER", "1") == "1"
    ats = [
        nc.declare_dram_parameter(f"at{g}", [128, KT, GROWS], at_dt, isOutput=False)
        for g in range(NG)
    ] if not _rg else None
    e2 = (nc.declare_dram_parameter("e2", [128, KT, D], e2_dt, isOutput=False)
          if not _rg else None)
    rowgather = _rg
    if rowgather:
        # A-rows per edge endpoint, fp8(A-0.5) viewed as int16 k-pairs;
        # either gathered on device (transpose dma_gather) or pre-gathered
        # on host (rg_pre, the sharding hint's "[B/M, N] blocks").
        rg_pre = os.environ.get("RG_PRE", "1") == "1"
        rg_dr = os.environ.get("RG_DR", "1") == "1"
        if not rg_pre:
            afull = nc.declare_dram_parameter(
                "afull", [N, N // 2], mybir.dt.int16, isOutput=False)
            ridx16 = nc.declare_dram_parameter(
                "ridx16", [128, 2 * EPC // 16], mybir.dt.int16, isOutput=False)
        e2dr = nc.declare_dram_parameter(
            "e2dr", [128, KT // 2, 2, D], fp8, isOutput=False)
        if rg_pre:
            # host pre-gathers the [2*EPC, N] A-row blocks (the sharding hint's
            # "gathered [B/M, N] blocks") chunk-major so each 1-tile chunk is a
            # contiguous 1MB sequential load on the sync HWDGE ring.
            if rg_dr:
                # plane-separated pairs: plain DoubleRow, natural AP slices
                arows = nc.declare_dram_parameter(
                    "arows", [128, 2 * EPC // 128, KT // 2, 2, 128],
                    fp8, isOutput=False)
            else:
                arows = nc.declare_dram_parameter(
                    "arows", [128, 2 * EPC // 128, KT // 2, 128],
                    mybir.dt.int16, isOutput=False)
    sidx16 = nc.declare_dram_parameter(
        "sidx16", [128, EPC // 16], mybir.dt.int16, isOutput=False)
    didx16 = nc.declare_dram_parameter(
        "didx16", [128, EPC // 16], mybir.dt.int16, isOutput=False)
    bias = nc.declare_dram_parameter("bias", [128, D], fp32, isOutput=False)
    scidx = nc.declare_dram_parameter("scidx", [128, MT], mybir.dt.int32, isOutput=False)
    hofs = nc.declare_dram_parameter("hofs", [1, 2], mybir.dt.int32, isOutput=False)
    sidx = nc.declare_dram_parameter("sidx", [128, JT], mybir.dt.int32, isOutput=False)
    didx = nc.declare_dram_parameter("didx", [128, JT], mybir.dt.int32, isOutput=False)
    out = nc.declare_dram_parameter("out", [128, JT], fp32, isOutput=True)

    with tile.TileContext(nc) as tc:
        with (
            tc.tile_pool(name="atp", bufs=1) as atp,
            tc.tile_pool(name="e2p", bufs=1) as e2p,
            tc.tile_pool(name="psum", bufs=MT, space="PSUM") as psum,
            tc.tile_pool(name="hwork", bufs=4) as hwork,
            tc.tile_pool(name="dram", bufs=1, space="DRAM") as dram,
            tc.tile_pool(name="const", bufs=1) as constp,
            tc.tile_pool(name="gat", bufs=1) as gat,
            tc.tile_pool(name="small", bufs=1) as small,
        ):
            ag_mode = os.environ.get("AG_MODE", "cc2")
            use_scatter = ag_mode == "scatter"
            h_shard = dram.tile([ROWS, D], bf16)  # legacy modes stay bf16
            # uneven chunks (m-tiles 0-1 / 2-3 / 4-7): the first small AG
            # starts right after group 0 so the CC pipeline opens ~14us
            # earlier; the last doorbell time is unchanged.
            CH_MT = [int(x) for x in os.environ.get("AG_CHUNKS", "8").split(",")]
            assert sum(CH_MT) == MT, CH_MT
            CH_OF = [sum(CH_MT[:i]) for i in range(len(CH_MT))]   # first m-tile
            CH_TRG = [o + n - 1 for o, n in zip(CH_OF, CH_MT)]    # last m-tile
            h_fp8 = os.environ.get("H_FP8", "0") == "1"
            h_dt = fp8 if h_fp8 else bf16
            h_shg = [
                dram.tile([n * 128, D], h_dt, name=f"h_shg{i}")
                for i, n in enumerate(CH_MT)
            ]
            if os.environ.get("H_SHARED") == "1":
                h_space = "Shared"
            else:
                h_space = "Local" if (ag_mode == "cc2" or os.environ.get("H_LOCAL") == "1") else "Shared"
            h_full = dram.tile([N, D], h_dt, addr_space=h_space)
            bar_in = dram.tile([8, 16], fp32)
            bar_out = dram.tile([8 * N_CORES, 16], fp32, addr_space=h_space)

            # Small latency-critical loads FIRST so they never queue behind
            # the 16MB A stream on the sync HWDGE ring.
            bias_sb = constp.tile([128, D], fp32)
            if not (rowgather and os.environ.get("RG_PRE", "1") == "1"):
                nc.sync.dma_start(out=bias_sb[:], in_=bias[:])
            if os.environ.get("SWDGE_WARM", "0") == "1":
                swarm = constp.tile([1, 16], fp32)
                nc.gpsimd.dma_start(out=swarm[:], in_=bias[0:1, 0:16])
            if use_scatter or os.environ.get("CC_WARM", "0") == "1":
                nc.sync.dma_start(out=bar_in[:], in_=bias_sb[0:8, 0:16])
            if os.environ.get("CC_WARM", "0") == "1":
                # tiny AllReduce issued before the matmuls: absorbs the first
                # collective's ~40us cold-start (uCode fetch / mesh program
                # load) so the real AllGathers run at the chained rate.
                nc.gpsimd.collective_compute(
                    "AllReduce",
                    mybir.AluOpType.add,
                    replica_groups=[list(range(N_CORES))],
                    ins=[bar_in[:].opt()],
                    outs=[bar_in[:].opt()],
                )
            use_dg = os.environ.get("GATHER_MODE", "dg") == "dg"
            sidx_sb = constp.tile([128, JT], mybir.dt.int32)
            didx_sb = constp.tile([128, JT], mybir.dt.int32)
            if not use_dg:
                nc.sync.dma_start(out=sidx_sb[:], in_=sidx[:])
                nc.sync.dma_start(out=didx_sb[:], in_=didx[:])
            scidx_sb = constp.tile([128, MT], mybir.dt.int32)
            hofs_sb = constp.tile([1, 2], mybir.dt.int32)
            sidx16_sb = constp.tile([128, EPC // 16], mybir.dt.int16)
            didx16_sb = constp.tile([128, EPC // 16], mybir.dt.int16)

            # Batched loads: few big DMAs. Small leading chunks so the first
            # matmuls start early; e2 and group-0 A interleave so k-tiles
            # arrive in lockstep. All bounds even so fp8 DoubleRow k-pairs
            # never straddle a chunk boundary.
            AT0_BOUNDS = [0, 2, 6, 14, 30, 64]
            E2_BOUNDS = [0, 2, 6, 14, 30, 64]
            ATN_BOUNDS = [0, 22, 44, 64] if mm_fp8 else [0, 22, 43, 64]
            at_t = [[None] * KT for _ in range(NG)]  # [group][k] -> AP [128, GROWS]
            e2_t = [None] * KT
            # fp8 pair APs: [group][k//2] -> AP [128, 2, GROWS] / [128, 2, D]
            at_p = [[None] * (KT // 2) for _ in range(NG)]
            e2_p = [None] * (KT // 2)

            def load_e2(lo, hi):
                if rowgather:
                    return
                ec = e2p.tile(
                    [128, hi - lo, D], e2_dt, name=f"e2c_{lo}", tag=f"e2c{lo}")
                nc.sync.dma_start(out=ec[:], in_=e2[:, lo:hi, :])
                for k in range(lo, hi):
                    e2_t[k] = ec[:, k - lo, :]
                if mm_fp8:
                    for k in range(lo, hi, 2):
                        e2_p[k // 2] = ec[:, k - lo:k - lo + 2, :]

            def load_at(g, lo, hi):
                if rowgather:
                    return
                ac = atp.tile(
                    [128, hi - lo, GROWS], at_dt if mm_fp8 else bf16,
                    name=f"atc_{g}_{lo}", tag=f"atc{g}_{lo}",
                )
                if a_fp8 and not mm_fp8:
                    # SWDGE cast-DMA: fp8 in HBM -> bf16 in SBUF (halves HBM read)
                    nc.gpsimd.dma_start(out=ac[:], in_=ats[g][:, lo:hi, :])
                else:
                    nc.sync.dma_start(out=ac[:], in_=ats[g][:, lo:hi, :])
                for k in range(lo, hi):
                    at_t[g][k] = ac[:, k - lo, :]
                if mm_fp8:
                    for k in range(lo, hi, 2):
                        at_p[g][k // 2] = ac[:, k - lo:k - lo + 2, :]

            for ci in range(len(AT0_BOUNDS) - 1):
                load_e2(E2_BOUNDS[ci], E2_BOUNDS[ci + 1])
                load_at(0, AT0_BOUNDS[ci], AT0_BOUNDS[ci + 1])
                if ci == 0:
                    if use_dg and not rowgather:
                        nc.sync.dma_start(out=sidx16_sb[:], in_=sidx16[:])
                        nc.sync.dma_start(out=didx16_sb[:], in_=didx16[:])
                    if os.environ.get("AG_MODE", "cc2") == "scatter":
                        nc.sync.dma_start(out=scidx_sb[:], in_=scidx[:])
                        nc.sync.dma_start(out=hofs_sb[:], in_=hofs[:])
            for g in range(1, NG):
                for ci in range(len(ATN_BOUNDS) - 1):
                    load_at(g, ATN_BOUNDS[ci], ATN_BOUNDS[ci + 1])

            hs_all = gat.tile([128, JT, D], h_dt, name="hs_all", tag="hs_all")
            hd_all = gat.tile([128, JT, D], h_dt, name="hd_all", tag="hd_all")

            # tiny early sqrt: hoists the Sqrt ACT-table load out of the
            # edge-phase critical tail (Square alone would pick a table set
            # without Sqrt, forcing a 1.3us reload right before the output)
            warm = small.tile([128, 2], fp32, name="warm", tag="warm")
            nc.scalar.sqrt(warm[:, 0:1], bias_sb[:, 0:1])
            # node norms: ss/st/inv columns per m-tile
            ss = small.tile([128, MT], fp32, name="ss", tag="ss")
            st = small.tile([128, MT], fp32, name="st", tag="st")
            inv = small.tile([128, MT], fp32, name="inv", tag="inv")
            out_sb = constp.tile([128, JT], fp32)

            scatters = []
            hb_all = gat.tile([128, MT, D], h_dt, name="hb_all", tag="hb_all")
            if rowgather:
                NT = 2 * EPC // 128          # 16 node tiles (src || dst)
                RG_CHUNKS = [int(x) for x in os.environ.get(
                    "RG_CHUNKS", "1,1,1,1,1,1,1,1,1,1,1,1,1,1,1,1").split(",")]
                assert sum(RG_CHUNKS) == NT, RG_CHUNKS
                NCH = len(RG_CHUNKS)
                KT16 = KT // 2               # 32 int16 k-planes
                E2_SPLITS = ((0, 2), (2, 8), (8, 16), (16, 24), (24, KT16))
                e2dr_sbs = [
                    e2p.tile([128, hi - lo, 2, D], fp8, name=f"e2dr_sb{lo}")
                    for lo, hi in E2_SPLITS
                ]

                def e2p_slice(t):
                    for (lo, hi), tl in zip(E2_SPLITS, e2dr_sbs):
                        if lo <= t < hi:
                            return tl[:, t - lo, :, :]
                ridx_sb = constp.tile([128, 2 * EPC // 16], mybir.dt.int16)
                # ridx first (gates the first gather); bulk e2dr on the
                # scalar ring so it never queues ahead of it.
                if not rg_pre:
                    st_eng.dma_start(out=ridx_sb[:], in_=ridx16[:])
                # e2dr on the scalar ring, piece 0 first; the big pieces are
                # issued after chunks 0/1 below so they don't race the loads
                # that gate the first matmuls for the shared DMA engines.
                st_eng.dma_start(
                    out=e2dr_sbs[0][:], in_=e2dr[:, 0:2, :, :])
                if not rg_pre:
                    st_eng.dma_start(out=e2dr_sbs[1][:], in_=e2dr[:, 2:8, :, :])
                    st_eng.dma_start(out=e2dr_sbs[2][:], in_=e2dr[:, 8:KT16, :, :])
                rg_dot = small.tile([128, JT], fp32, name="rgdot", tag="rgdot")
                rg_ns = small.tile([128, JT], fp32, name="rgns", tag="rgns")
                rg_nd = small.tile([128, JT], fp32, name="rgnd", tag="rgnd")
                rg_edt = bf16 if os.environ.get("EDGE_BF16", "1") == "1" else fp32
                rg_prod = gat.tile([128, JT, D], rg_edt, name="rgprod", tag="rgprod")
                rg_sqs = gat.tile([128, JT, D], rg_edt, name="rgsqs", tag="rgsqs")
                rg_sqd = gat.tile([128, JT, D], rg_edt, name="rgsqd", tag="rgsqd")
                with nc.named_scope("matmul"):
                    ps_t = [
                        psum.tile([128, D], fp32, name=f"ps_{m}", tag="ps")
                        for m in range(min(MT, NT))
                    ]
                    ch_of = [sum(RG_CHUNKS[:i]) for i in range(NCH)]
                    for ch in range(NCH):
                        CT = RG_CHUNKS[ch]
                        npc = CT * 128       # idxs per chunk
                        if rg_pre and rg_dr:
                            ag_sb = hwork.tile(
                                [128, KT16, 2, npc], fp8,
                                name=f"ag_{ch}", tag=f"ag{CT}")
                        else:
                            ag_sb = hwork.tile(
                                [128, KT16, npc], mybir.dt.int16,
                                name=f"ag_{ch}", tag=f"ag{CT}")
                        if rg_pre:
                            assert CT == 1, "rg_pre ships 1-tile chunks"
                            nc.sync.dma_start(
                                out=ag_sb[:],
                                in_=(arows[:, ch_of[ch], :, :, :] if rg_dr
                                     else arows[:, ch_of[ch], :, :]))
                            if ch == 0:
                                # all e2dr pieces + bias on the scalar ring,
                                # emitted BEFORE chunk 0's matmuls (which
                                # consume every k-plane) so the reads have
                                # real sync deps; fine-grained pieces keep
                                # the per-plane wait short.  chunk 0 stays
                                # the sync ring's first transfer.
                                for pi, (lo, hi) in enumerate(E2_SPLITS):
                                    if pi == 0:
                                        continue
                                    st_eng.dma_start(
                                        out=e2dr_sbs[pi][:],
                                        in_=e2dr[:, lo:hi, :, :])
                                st_eng.dma_start(
                                    out=bias_sb[:], in_=bias[:])
                        else:
                            cs = slice(ch_of[ch] * 8, (ch_of[ch] + CT) * 8)
                            nc.gpsimd.dma_gather(
                                ag_sb[:, :, :], afull[:], ridx_sb[:, cs],
                                npc, npc, N // 2, transpose=True)
                        agf = (ag_sb[:] if (rg_pre and rg_dr)
                               else ag_sb[:].bitcast(fp8))
                        if os.environ.get("RG_NOMM") == "1":
                            for lt in range(CT):
                                nt = ch_of[ch] + lt
                                dst = (hs_all[:, nt, :] if nt < JT
                                       else hd_all[:, nt - JT, :])
                                nc.vector.tensor_scalar(
                                    out=dst,
                                    in0=agf[:, 0, lt * 256:lt * 256 + 256],
                                    scalar1=1.0, scalar2=None,
                                    op0=mybir.AluOpType.mult)
                            continue
                        for lt in range(CT):  # matmul chains
                            nt = ch_of[ch] + lt
                            ps = ps_t[nt % len(ps_t)]
                            for t in range(KT16):
                                if rg_pre and rg_dr:
                                    # plane-separated pairs: natural slice,
                                    # plain DoubleRow
                                    lhs_ap = agf[:, t, :, lt * 128:(lt + 1) * 128]
                                    pm = mybir.MatmulPerfMode.DoubleRow
                                else:
                                    # interleaved stationary: plane stride 1
                                    # (the A/B byte pair), column stride 2,
                                    # columns shipped host-reversed
                                    # (SwInterleave contract)
                                    lhs_ap = bass.AP(
                                        agf.tensor,
                                        agf.offset + t * (2 * npc) + lt * 256,
                                        [agf.ap[0], [1, 2], [2, 128]],
                                    )
                                    pm = mybir.MatmulPerfMode.DoubleRowSwInterleave
                                nc.tensor.matmul(
                                    out=ps[:],
                                    lhsT=lhs_ap,
                                    rhs=e2p_slice(t),
                                    start=(t == 0),
                                    stop=(t == KT16 - 1),
                                    perf_mode=pm,
                                )
                            dst = (hs_all[:, nt // 2, :] if nt % 2 == 0
                                   else hd_all[:, nt // 2, :])
                            nc.vector.tensor_tensor(
                                out=dst, in0=ps[:], in1=bias_sb[:],
                                op=mybir.AluOpType.add,
                            )
                            if nt % 2 == 1:
                                jb = nt // 2
                                hs_b = hs_all[:, jb, :]
                                hd_b = hd_all[:, jb, :]
                                nc.vector.tensor_tensor(
                                    out=rg_prod[:, jb, :], in0=hs_b, in1=hd_b,
                                    op=mybir.AluOpType.mult)
                                nc.scalar.square(rg_sqs[:, jb, :], hs_b)
                                nc.scalar.square(rg_sqd[:, jb, :], hd_b)
                                nc.vector.tensor_reduce(
                                    out=rg_dot[:, jb:jb + 1],
                                    in_=rg_prod[:, jb, :],
                                    axis=mybir.AxisListType.X,
                                    op=mybir.AluOpType.add)
                                nc.vector.tensor_reduce(
                                    out=rg_ns[:, jb:jb + 1],
                                    in_=rg_sqs[:, jb, :],
                                    axis=mybir.AxisListType.X,
                                    op=mybir.AluOpType.add)
                                nc.vector.tensor_reduce(
                                    out=rg_nd[:, jb:jb + 1],
                                    in_=rg_sqd[:, jb, :],
                                    axis=mybir.AxisListType.X,
                                    op=mybir.AluOpType.add)
                with nc.named_scope("rgchain"):
                    nsnd = small.tile([128, JT], fp32, name="nsnd", tag="nsnd")
                    nc.vector.tensor_tensor(
                        out=nsnd[:], in0=rg_ns[:], in1=rg_nd[:],
                        op=mybir.AluOpType.mult)
                    stq = small.tile([128, JT], fp32, name="stq", tag="stq")
                    nc.scalar.sqrt(stq[:], nsnd[:])
                    invq = small.tile([128, JT], fp32, name="invq", tag="invq")
                    nc.vector.reciprocal(invq[:], stq[:])
                    ad = small.tile([128, JT], fp32, name="ad", tag="ad")
                    nc.vector.tensor_scalar(
                        out=ad[:].bitcast(mybir.dt.uint32),
                        in0=rg_dot[:].bitcast(mybir.dt.uint32),
                        scalar1=0x7FFFFFFF, scalar2=None,
                        op0=mybir.AluOpType.bitwise_and,
                    )
                    nc.vector.tensor_tensor(
                        out=out_sb[:], in0=ad[:], in1=invq[:],
                        op=mybir.AluOpType.mult)
            else:
              with nc.named_scope("matmul"):
                ps_t = [
                    psum.tile([128, D], fp32, name=f"ps_{m}", tag="ps")
                    for m in range(MT)
                ]
                ag_chunks = []
                for g in range(NG):
                    ms = range(g * MPG, (g + 1) * MPG)
                    # last group runs m-outer so m6's psum (and its store)
                    # completes ~11us before the final matmul, shortening the
                    # last AllGather's doorbell chain to m7's store alone
                    if mm_fp8:
                        KP = KT // 2
                        order = (
                            [(kp, m) for m in ms for kp in range(KP)]
                            if g == NG - 1 else
                            [(kp, m) for kp in range(KP) for m in ms]
                        )
                        for kp, m in order:
                            lm = m - g * MPG
                            nc.tensor.matmul(
                                out=ps_t[m][:],
                                lhsT=at_p[g][kp][:, :, lm * 128:(lm + 1) * 128],
                                rhs=e2_p[kp],
                                start=(kp == 0),
                                stop=(kp == KP - 1),
                                perf_mode=mybir.MatmulPerfMode.DoubleRow,
                            )
                    else:
                        order = (
                            [(k, m) for m in ms for k in range(KT)]
                            if g == NG - 1 else
                            [(k, m) for k in range(KT) for m in ms]
                        )
                        for k, m in order:
                            lm = m - g * MPG
                            nc.tensor.matmul(
                                out=ps_t[m][:],
                                lhsT=at_t[g][k][:, lm * 128:(lm + 1) * 128],
                                rhs=e2_t[k],
                                start=(k == 0),
                                stop=(k == KT - 1),
                            )
                    with nc.named_scope(f"norm{g}"):
                        for m in ms:
                            if use_norm:
                                t = hwork.tile([128, D], fp32, name=f"t_{m}", tag="t")
                                nc.vector.tensor_tensor(
                                    out=t[:], in0=ps_t[m][:], in1=bias_sb[:],
                                    op=mybir.AluOpType.add,
                                )
                                sq = hwork.tile([128, D], fp32, name=f"sq_{m}", tag="sq")
                                if os.environ.get("NORM_IMPL", "dve") == "act":
                                    nc.scalar.activation(
                                        out=sq[:], in_=t[:],
                                        func=mybir.ActivationFunctionType.Square,
                                        accum_out=ss[:, m:m + 1],
                                    )
                                else:
                                    nc.vector.tensor_tensor_reduce(
                                        out=sq[:], in0=t[:], in1=t[:],
                                        scale=1.0, scalar=0.0,
                                        op0=mybir.AluOpType.mult,
                                        op1=mybir.AluOpType.add,
                                        accum_out=ss[:, m:m + 1],
                                    )
                                nc.scalar.sqrt(st[:, m:m + 1], ss[:, m:m + 1])
                                nc.vector.reciprocal(inv[:, m:m + 1], st[:, m:m + 1])
                                hb = hb_all[:, m, :]
                                if os.environ.get("NORM_IMPL", "dve") == "act":
                                    nc.scalar.activation(
                                        out=hb, in_=t[:],
                                        func=mybir.ActivationFunctionType.Copy,
                                        bias=0.0, scale=inv[:, m:m + 1],
                                    )
                                else:
                                    nc.vector.tensor_scalar(
                                        out=hb, in0=t[:],
                                        scalar1=inv[:, m:m + 1], scalar2=None,
                                        op0=mybir.AluOpType.mult,
                                    )
                            else:
                                hb = hb_all[:, m, :]
                                if h_fp8:
                                    # h/4 in fp8e4; the global scale cancels
                                    # in cos = |dot|/(|hs||hd|).  bias_sb is
                                    # shipped pre-scaled by 1/4.
                                    nc.vector.scalar_tensor_tensor(
                                        out=hb, in0=ps_t[m][:], scalar=0.25,
                                        in1=bias_sb[:],
                                        op0=mybir.AluOpType.mult,
                                        op1=mybir.AluOpType.add,
                                    )
                                else:
                                    nc.vector.tensor_tensor(
                                        out=hb, in0=ps_t[m][:], in1=bias_sb[:],
                                        op=mybir.AluOpType.add,
                                    )
                            if ag_mode == "cc2":
                                ch = max(
                                    i for i, o in enumerate(CH_OF) if m >= o
                                )
                                lm2 = m - CH_OF[ch]
                                # SWDGE store: its completion sem does not
                                # alias the sync-ring A-load lanes, so the
                                # AllGather doorbell fires as soon as the
                                # chunk is really ready (was +18us late)
                                h_st = (nc.gpsimd if os.environ.get(
                                    "H_ENG", "gpsimd") == "gpsimd" else st_eng)
                                h_st.dma_start(
                                    out=h_shg[ch][lm2 * 128:(lm2 + 1) * 128, :],
                                    in_=hb,
                                )

                    last_m = g * MPG + MPG - 1
                    if ag_mode == "cc2" and last_m in CH_TRG:
                        ch = CH_TRG.index(last_m)
                        base = CH_OF[ch] * 128 * N_CORES
                        size = CH_MT[ch] * 128 * N_CORES
                        with nc.named_scope(f"ag{ch}"):
                            ag_chunks.append(nc.gpsimd.collective_compute(
                                "AllGather",
                                mybir.AluOpType.bypass,
                                replica_groups=[list(range(N_CORES))],
                                ins=[h_shg[ch][:]],
                                outs=[h_full[base:base + size, :]],
                            ))
                with nc.named_scope("allgather"):
                    if ag_mode == "cc2":
                        pass
                    elif use_scatter:
                        ofs = nc.sync.value_load(
                            hofs_sb[0:1, 0:1], min_val=0, max_val=(N_CORES - 1) * ROWS
                        )
                        hf = h_full[:]
                        dyn_out = bass.AP(
                            hf.tensor,
                            ofs * D,
                            [[D, 128], [128 * D, MT], [1, D]],
                        )
                        hw = nc.sync.dma_start(out=dyn_out, in_=hb_all[:])
                        if os.environ.get("NOBAR") == "1":
                            bar_cc = hw
                        else:
                            bar_cc = nc.gpsimd.collective_compute(
                                "AllReduce",
                                mybir.AluOpType.add,
                                replica_groups=[list(range(N_CORES))],
                                ins=[bar_in[:].opt()],
                                outs=[bar_in[:].opt()],
                            )
                            bass._add_dep_helper(
                                bar_cc.ins, hw.ins, sync=True,
                                reason="barrier waits for h write",
                            )
                    else:
                        for m in range(MT):
                            st_eng.dma_start(
                                out=h_shard[m * 128:(m + 1) * 128, :],
                                in_=hb_all[:, m, :],
                            )
                        bar_cc = nc.gpsimd.collective_compute(
                            "AllGather",
                            mybir.AluOpType.bypass,
                            replica_groups=[list(range(N_CORES))],
                            ins=[h_shard[:]],
                            outs=[h_full[:]],
                        )

            # SWDGE descriptor generation for the edge-row gathers.  Issued
            # after the collective doorbells (so the h_full RAW edge keeps
            # its normal writer->reader direction) but the desc-gen itself
            # (~1.3us/call on the Q7) runs under the AllGather's mesh
            # rendezvous; trigger_dma in the edges scope then only pays the
            # DMA transfer.
            QN = int(os.environ.get("GATHER_SPLIT", "1"))
            use_dg = os.environ.get("GATHER_MODE", "dg") == "dg"
            if prep_gather and use_dg:
                assert QN == 1, "prep_gather path gathers each side whole"
                gprep = nc.alloc_semaphore("gprep")
                gsem_s = nc.alloc_semaphore("gath_dma_s")
                gsem_d = nc.alloc_semaphore("gath_dma_d")
                nc.gpsimd.dma_gather(
                    hs_all[:, :, :], h_full[:], sidx16_sb[:, :], EPC, EPC, D,
                    prepare_only=True, sem=gsem_s,
                    queue_num=int(os.environ.get("PREP_Q", "0")),
                ).then_inc(gprep, 1)
                nc.gpsimd.dma_gather(
                    hd_all[:, :, :], h_full[:], didx16_sb[:, :], EPC, EPC, D,
                    prepare_only=True, sem=gsem_d,
                    queue_num=int(os.environ.get("PREP_Q", "0")),
                ).then_inc(gprep, 1)

            with nc.named_scope("edges"):
                HEPC = EPC // QN
                HJT = JT // QN
                gs = []
                interleave = (
                    use_dg and not prep_gather and QN == 1 and not use_norm
                    and os.environ.get("EDGE_TTR", "0") != "1"
                    and os.environ.get("EDGE_IL", "1") == "1"
                ) or rowgather
                if interleave:
                    pass  # gathers emitted inside the math below
                elif use_dg:
                    if prep_gather:
                        # Manual protocol (docstring pattern): wait for the
                        # descriptor writes, fire both queues' entries once
                        # the AllGather has fully written h_full, then gate
                        # the consumer engines on the DMA-completion sems.
                        nc.gpsimd.wait_ge(gprep, 2)
                        trig = nc.gpsimd.trigger_dma(
                            count=2, queue_num=int(os.environ.get("PREP_Q", "0")))
                        if ag_chunks:
                            bass._add_dep_helper(
                                trig.ins, ag_chunks[-1].ins, sync=True,
                                reason="gather transfers wait for h_full")
                        nc.vector.wait_ge(gsem_s, 16)
                        nc.vector.wait_ge(gsem_d, 16)
                        nc.scalar.wait_ge(gsem_s, 16)
                        nc.scalar.wait_ge(gsem_d, 16)
                    else:
                        for h in range(QN):
                            js = slice(h * HJT, (h + 1) * HJT)
                            cs = slice(h * (HEPC // 16), (h + 1) * (HEPC // 16))
                            gs.append(nc.gpsimd.dma_gather(
                                hs_all[:, js, :], h_full[:], sidx16_sb[:, cs],
                                HEPC, HEPC, D))
                            gs.append(nc.gpsimd.dma_gather(
                                hd_all[:, js, :], h_full[:], didx16_sb[:, cs],
                                HEPC, HEPC, D))
                        if use_scatter:
                            for gg in gs:
                                bass._add_dep_helper(
                                    gg.ins, bar_cc.ins, sync=True,
                                    reason="gathers wait for cross-core barrier")
                else:
                    _ind_gathers(nc, bass, hs_all, hd_all, h_full, sidx_sb, didx_sb)
                dot = small.tile([128, JT], fp32, name="dot", tag="dot")

                if rowgather:
                    pass
                elif interleave:
                    # hs-side square/reduce runs under the hd gather's Q7
                    # descriptor generation (~8.5us), so only the dot/nd
                    # chain remains after the second gather lands.
                    e_dt = bf16 if os.environ.get("EDGE_BF16", "1") == "1" else fp32
                    ns = small.tile([128, JT], fp32, name="ns", tag="ns")
                    nd = small.tile([128, JT], fp32, name="nd", tag="nd")
                    prod = gat.tile([128, JT, D], e_dt, name="prod", tag="prod")
                    sq_s = gat.tile([128, JT, D], e_dt, name="sq_s", tag="sq_s")
                    sq_d = gat.tile([128, JT, D], e_dt, name="sq_d", tag="sq_d")
                    if not rowgather:
                        nc.gpsimd.dma_gather(
                            hs_all[:, :, :], h_full[:], sidx16_sb[:, :],
                            EPC, EPC, D)
                    nc.scalar.square(sq_s[:], hs_all[:])
                    nc.vector.tensor_reduce(
                        out=ns[:], in_=sq_s[:],
                        axis=mybir.AxisListType.X, op=mybir.AluOpType.add)
                    if not rowgather:
                        nc.gpsimd.dma_gather(
                            hd_all[:, :, :], h_full[:], didx16_sb[:, :],
                            EPC, EPC, D)
                    nc.vector.tensor_tensor(
                        out=prod[:], in0=hs_all[:], in1=hd_all[:],
                        op=mybir.AluOpType.mult)
                    nc.scalar.square(sq_d[:], hd_all[:])
                    nc.vector.tensor_reduce(
                        out=dot[:], in_=prod[:],
                        axis=mybir.AxisListType.X, op=mybir.AluOpType.add)
                    nc.vector.tensor_reduce(
                        out=nd[:], in_=sq_d[:],
                        axis=mybir.AxisListType.X, op=mybir.AluOpType.add)
                    nsnd = small.tile([128, JT], fp32, name="nsnd", tag="nsnd")
                    nc.vector.tensor_tensor(
                        out=nsnd[:], in0=ns[:], in1=nd[:],
                        op=mybir.AluOpType.mult)
                    stq = small.tile([128, JT], fp32, name="stq", tag="stq")
                    nc.scalar.sqrt(stq[:], nsnd[:])
                    invq = small.tile([128, JT], fp32, name="invq", tag="invq")
                    nc.vector.reciprocal(invq[:], stq[:])
                    ad = small.tile([128, JT], fp32, name="ad", tag="ad")
                    nc.vector.tensor_scalar(
                        out=ad[:].bitcast(mybir.dt.uint32),
                        in0=dot[:].bitcast(mybir.dt.uint32),
                        scalar1=0x7FFFFFFF, scalar2=None,
                        op0=mybir.AluOpType.bitwise_and,
                    )
                    nc.vector.tensor_tensor(
                        out=out_sb[:], in0=ad[:], in1=invq[:],
                        op=mybir.AluOpType.mult)
                elif os.environ.get("EDGE_TTR", "0") == "1" and not use_norm:
                    # dot/ns/nd fused multiply-accumulates, one [128, D] slab
                    # per edge block, spread across DVE (dot), ACT (ns) and
                    # GpSimd (nd) so the three reductions run in parallel.
                    ns = small.tile([128, JT], fp32, name="ns", tag="ns")
                    nd = small.tile([128, JT], fp32, name="nd", tag="nd")
                    for j in range(JT):
                        pw = hwork.tile([128, D], fp32, name=f"pw_{j}", tag="pw")
                        nc.vector.tensor_tensor_reduce(
                            out=pw[:], in0=hs_all[:, j, :], in1=hd_all[:, j, :],
                            scale=1.0, scalar=0.0,
                            op0=mybir.AluOpType.mult,
                            op1=mybir.AluOpType.add,
                            accum_out=dot[:, j:j + 1])
                        qw = hwork.tile([128, D], fp32, name=f"qw_{j}", tag="qw")
                        nc.scalar.activation(
                            out=qw[:], in_=hs_all[:, j, :],
                            func=mybir.ActivationFunctionType.Square,
                            accum_out=ns[:, j:j + 1])
                        rw = hwork.tile([128, D], fp32, name=f"rw_{j}", tag="rw")
                        nc.vector.tensor_tensor_reduce(
                            out=rw[:], in0=hd_all[:, j, :], in1=hd_all[:, j, :],
                            scale=1.0, scalar=0.0,
                            op0=mybir.AluOpType.mult,
                            op1=mybir.AluOpType.add,
                            accum_out=nd[:, j:j + 1])
                    nsnd = small.tile([128, JT], fp32, name="nsnd", tag="nsnd")
                    nc.vector.tensor_tensor(
                        out=nsnd[:], in0=ns[:], in1=nd[:],
                        op=mybir.AluOpType.mult)
                    stq = small.tile([128, JT], fp32, name="stq", tag="stq")
                    nc.scalar.sqrt(stq[:], nsnd[:])
                    invq = small.tile([128, JT], fp32, name="invq", tag="invq")
                    nc.vector.reciprocal(invq[:], stq[:])
                    ad = small.tile([128, JT], fp32, name="ad", tag="ad")
                    nc.vector.tensor_scalar(
                        out=ad[:].bitcast(mybir.dt.uint32),
                        in0=dot[:].bitcast(mybir.dt.uint32),
                        scalar1=0x7FFFFFFF, scalar2=None,
                        op0=mybir.AluOpType.bitwise_and,
                    )
                    nc.vector.tensor_tensor(
                        out=out_sb[:], in0=ad[:], in1=invq[:],
                        op=mybir.AluOpType.mult)
                elif use_norm and os.environ.get("EDGE_IMPL", "new") == "new":
                    for j in range(JT):
                        prod = hwork.tile([128, D], fp32, name=f"prod_{j}", tag="prod")
                        nc.vector.tensor_tensor_reduce(
                            out=prod[:],
                            in0=hs_all[:, j, :],
                            in1=hd_all[:, j, :],
                            scale=1.0,
                            scalar=0.0,
                            op0=mybir.AluOpType.mult,
                            op1=mybir.AluOpType.add,
                            accum_out=dot[:, j:j + 1],
                        )
                    nc.scalar.activation(
                        out=out_sb[:], in_=dot[:],
                        func=mybir.ActivationFunctionType.Abs,
                    )
                else:
                    ns = small.tile([128, JT], fp32, name="ns", tag="ns")
                    nd = small.tile([128, JT], fp32, name="nd", tag="nd")
                    # bf16 intermediates double DVE throughput; the reduces
                    # still accumulate into fp32 (dot/ns/nd), so only the
                    # per-element products are rounded (~2^-8 rel, harmless
                    # next to the fp8 input quantization).
                    e_dt = bf16 if os.environ.get("EDGE_BF16", "1") == "1" else fp32
                    prod = gat.tile([128, JT, D], e_dt, name="prod", tag="prod")
                    sq_s = gat.tile([128, JT, D], e_dt, name="sq_s", tag="sq_s")
                    sq_d = gat.tile([128, JT, D], e_dt, name="sq_d", tag="sq_d")
                    for h in range(QN):
                        js = slice(h * HJT, (h + 1) * HJT)
                        nc.vector.tensor_tensor(
                            out=prod[:, js, :], in0=hs_all[:, js, :],
                            in1=hd_all[:, js, :],
                            op=mybir.AluOpType.mult,
                        )
                        nc.vector.tensor_reduce(
                            out=dot[:, js], in_=prod[:, js, :],
                            axis=mybir.AxisListType.X,
                            op=mybir.AluOpType.add,
                        )
                        nc.scalar.square(sq_s[:, js, :], hs_all[:, js, :])
                        nc.scalar.square(sq_d[:, js, :], hd_all[:, js, :])
                        nc.vector.tensor_reduce(
                            out=ns[:, js], in_=sq_s[:, js, :],
                            axis=mybir.AxisListType.X,
                            op=mybir.AluOpType.add,
                        )
                        nc.vector.tensor_reduce(
                            out=nd[:, js], in_=sq_d[:, js, :],
                            axis=mybir.AxisListType.X,
                            op=mybir.AluOpType.add,
                        )
                        nsnd = small.tile([128, JT], fp32, name="nsnd", tag="nsnd")
                        nc.vector.tensor_tensor(
                            out=nsnd[:, js], in0=ns[:, js], in1=nd[:, js],
                            op=mybir.AluOpType.mult,
                        )
                        stq = small.tile([128, JT], fp32, name="stq", tag="stq")
                        nc.scalar.sqrt(stq[:, js], nsnd[:, js])
                        invq = small.tile([128, JT], fp32, name="invq", tag="invq")
                        nc.vector.reciprocal(invq[:, js], stq[:, js])
                        ad = small.tile([128, JT], fp32, name="ad", tag="ad")
                        nc.vector.tensor_scalar(
                            out=ad[:, js].bitcast(mybir.dt.uint32),
                            in0=dot[:, js].bitcast(mybir.dt.uint32),
                            scalar1=0x7FFFFFFF, scalar2=None,
                            op0=mybir.AluOpType.bitwise_and,
                        )
                        nc.vector.tensor_tensor(
                            out=out_sb[:, js], in0=ad[:, js], in1=invq[:, js],
                            op=mybir.AluOpType.mult,
                        )

            st_eng.dma_start(out=out[:], in_=out_sb[:])

    nc.compile()
    return nc


def _get_nc():
    if "nc" not in _CACHE:
        _CACHE["nc"] = _build()
    return _CACHE["nc"]


def _remap(n):
    # node id -> h_full row. cc2 mode: two chunked AllGathers; chunk g holds
    # rows [g*512, (g+1)*512) of every core shard, concatenated rank-major.
    if os.environ.get("AG_MODE", "cc2") != "cc2":
        return n
    ch_mt = [int(x) for x in os.environ.get("AG_CHUNKS", "8").split(",")]
    ch_of = np.array([sum(ch_mt[:i]) for i in range(len(ch_mt))]) * 128
    ch_rows = np.array(ch_mt) * 128
    o = n // ROWS
    l = n % ROWS
    g = np.searchsorted(ch_of, l, side="right") - 1
    return ch_of[g] * N_CORES + o * ch_rows[g] + (l - ch_of[g])


def kernel(edges, A_s, emb, Ws, bs):
    global LAST_RESULTS
    from concourse.bass_utils import run_bass_kernel_spmd

    bf16 = ml_dtypes.bfloat16
    A = np.asarray(A_s, dtype=np.float32)
    E = np.asarray(emb, dtype=np.float32)
    W = np.asarray(Ws, dtype=np.float32)
    b = np.asarray(bs, dtype=np.float32)
    ed = np.asarray(edges)

    a_fp8 = os.environ.get("A_FP8", "0") == "1"
    mm_fp8 = os.environ.get("MM_FP8", "1") == "1"
    shift_a = a_fp8 or mm_fp8
    f8 = ml_dtypes.float8_e4m3fn
    M = W[0].T @ W[1].T @ W[2].T                      # [D, D]
    e2_np_dt = f8 if mm_fp8 else bf16
    E2f = (E @ M).astype(e2_np_dt)                    # [N, D] as used on device
    # partition-major: [128(p), KT(t), D] with row t*128+p at [p, t, :]
    E2 = np.ascontiguousarray(E2f.reshape(KT, 128, D).transpose(1, 0, 2))
    b_eff = (b[0] @ W[1].T + b[1]) @ W[2].T + b[2]    # [D]
    if shift_a:
        # A shipped as fp8(A - 0.5); fold the +0.5 row-sum term into the bias
        b_eff = b_eff + 0.5 * E2f.astype(np.float32).sum(0)
    if os.environ.get("H_FP8", "0") == "1":
        b_eff = b_eff * 0.25
    bias_rep = np.ascontiguousarray(
        np.broadcast_to(b_eff.astype(np.float32), (128, D))
    )

    rowgather = os.environ.get("ROWGATHER", "1") == "1"
    if rowgather:
        A8 = (A - 0.5).astype(f8)                     # [N, N]
        afull16 = A8.view(np.int16)                   # [N, N//2] k-pairs
        E2q32 = E2f.astype(np.float32)
        e2dr = np.ascontiguousarray(
            E2q32.astype(f8).reshape(KT // 2, 128, 2, D)
            .transpose(1, 0, 2, 3))
    in_maps = []
    perms = []
    for c in range(N_CORES):
        m = {"bias": bias_rep}
        if not rowgather:
            m["e2"] = E2
        if not rowgather:
            for g in range(NG):
                r0 = c * ROWS + g * GROWS
                blk = A[r0:r0 + GROWS, :].T               # [N, GROWS]
                blk = (blk - 0.5).astype(f8) if shift_a else blk.astype(bf16)
                m[f"at{g}"] = np.ascontiguousarray(
                    blk.reshape(KT, 128, GROWS).transpose(1, 0, 2)
                )
        e = ed[c * EPC:(c + 1) * EPC].astype(np.int64)
        if rowgather:
            order = np.argsort(e[:, 0], kind="stable")
            perms.append(order)
            e = e[order]
            m["e2dr"] = e2dr
            # blocks interleaved [src_0, dst_0, src_1, dst_1, ...] so each
            # chunk completes whole edge blocks (per-chunk dot/norm math)
            flat = np.stack(
                [e[:, 0].reshape(JT, 128), e[:, 1].reshape(JT, 128)],
                axis=1).reshape(-1)
            # reverse within each 128-block: DoubleRowSwInterleave reads the
            # stationary columns last-first, so ship them pre-reversed.
            rev = flat.reshape(-1, 128)[:, ::-1].reshape(-1)
            if os.environ.get("RG_PRE", "1") == "1":
                # host pre-gather: the sharding hint's "[B/M, N] blocks",
                # chunk-major so each device load is contiguous
                if os.environ.get("RG_DR", "1") == "1":
                    g8 = A8[flat]                     # [2*EPC, N] fp8
                    m["arows"] = np.ascontiguousarray(
                        g8.reshape(2 * EPC // 128, 128, KT // 2, 128, 2)
                        .transpose(3, 0, 2, 4, 1))    # [p, ch, t, pair, j]
                else:
                    g = afull16[rev]                  # [2*EPC, N//2] int16
                    m["arows"] = np.ascontiguousarray(
                        g.reshape(2 * EPC // 128, 128, KT // 2, 128)
                        .transpose(3, 0, 2, 1))       # [p, ch, t, j]
            else:
                m["afull"] = afull16
                m["ridx16"] = np.ascontiguousarray(
                    np.tile(rev.astype(np.int16).reshape(-1, 16).T, (8, 1)))
        m["sidx"] = np.ascontiguousarray(
            _remap(e[:, 0]).astype(np.int32).reshape(JT, 128).T
        )
        dsrc = e[:, 0] if os.environ.get("PROBE_DD_EQ_SS") == "1" else e[:, 1]
        m["didx"] = np.ascontiguousarray(
            _remap(dsrc).astype(np.int32).reshape(JT, 128).T
        )

        QN = int(os.environ.get("GATHER_SPLIT", "1"))

        def wrap16(flat):
            # QN independent gathers: wrap each EPC/QN-index chunk separately
            def w(f):
                buf = f.astype(np.int16).reshape(-1, 16).T
                return np.tile(buf, (8, 1))
            step = EPC // QN
            return np.ascontiguousarray(
                np.hstack([w(flat[q * step:(q + 1) * step]) for q in range(QN)]))

        m["hofs"] = np.array([[c * ROWS, 0]], dtype=np.int32)
        m["scidx"] = np.ascontiguousarray(
            (c * ROWS + np.arange(MT)[None, :] * 128
             + np.arange(128)[:, None]).astype(np.int32))
        m["sidx16"] = wrap16(_remap(e[:, 0]))
        m["didx16"] = wrap16(_remap(dsrc))
        in_maps.append(m)

    nc = _get_nc()
    kw = {}
    if os.environ.get("KERNEL_TRACE_KW"):
        import json
        kw = json.loads(os.environ["KERNEL_TRACE_KW"])
    res = run_bass_kernel_spmd(nc, in_maps, list(range(N_CORES)), **kw)
    LAST_RESULTS = res

    outs = []
    for c in range(N_CORES):
        oc = np.ascontiguousarray(res.results[c]["out"].T).reshape(-1)
        if rowgather:
            inv = np.empty_like(perms[c])
            inv[perms[c]] = np.arange(EPC)
            oc = oc[inv]
        outs.append(oc)
    out = np.concatenate(outs)
    return np.maximum(out, 0.0).astype(np.float32)

